# revision 21
# baseline (speedup 1.0000x reference)
"""Trainium2 Bass kernel: LookupTransformerBlock (block-causal sparse attention).

Reference semantics (B=4, T=784, D=768, H=12, Dh=64, d_ff=3072):
  x_aug = LN1(concat(memory[:, :T], x))              # [B, 2T, D]
  h     = LN_att(x_aug)
  qkv   = h @ w_qkv.T ; block-causal attention over frames of 196
  x2    = x_aug + attn_out
  out   = (x2 + FFN(LN2(x2)))[:, T:, :]

Sharding: 8 cores = (batch b in 0..3) x (query-half hf in 0..1); each core
computes its 392 output rows with K/V over all 1568 positions (data-parallel,
no collectives).  One SPMD program; per-core differences live in input data.

Layout decisions (driven by the DMA/engine-overhead analysis of the previous
version's trace — 500+ small DMAs and per-tile mask exps were the bottleneck):
  - every DRAM tensor is packed host-side so a handful of dma_starts move
    everything with multi-KB per-partition contiguous rows, weights and
    activations in bf16;
  - the host permutes key columns per core so the query slice is always
    columns [1176:1568) -> LN1 chunk-3 stats double as the query stats and
    q-tiles are plain slices;
  - the attention mask is folded into the score matmul: per-head K tiles
    carry two extra contraction rows (frame-B mask bias, frame-A correction)
    and per-head Q tiles the matching gate rows (1, 1_{query in frame A}).
    Exp is then mask-free, so score j-tiles are exp'd in pairs out of PSUM;
  - softmax denominators come from a ones column in V, leave PSUM through
    per-head row DMAs, hit one batched reciprocal_approx_fast, and return
    as PE ones-matmul broadcasts (no DRAM bounces anywhere);
  - the final output stays feature-major on device; the host transposes.
"""

import os
import sys
from contextlib import ExitStack

import numpy as np
import ml_dtypes

for _p in ("/opt/trn_rl_repo", os.path.expanduser("~/.axon_site/_ro/trn_rl_repo")):
    if os.path.isdir(_p) and _p not in sys.path:
        sys.path.append(_p)

import concourse.bass as bass
import concourse.bacc as bacc
import concourse.mybir as mybir
import concourse.tile as tile
from concourse.bass_utils import run_bass_kernel_spmd

F32 = mybir.dt.float32
F32R = mybir.dt.float32r
BF16 = mybir.dt.bfloat16
AF = mybir.ActivationFunctionType
ALU = mybir.AluOpType
NPBF16 = ml_dtypes.bfloat16

B = 4
T = 784
D = 768
L = 2 * T            # 1568
NQ = 392             # query rows per core (= LN chunk width)
Q0 = L - NQ          # queries always live at columns [1176:1568)
H = 12
DFF = 3072
NPATCH = 196
DC = D // 128        # 6
FT = DFF // 128      # 24
NJT = 13             # j-tiles over L (12 x 128 + 32)
JSZ = [128] * 12 + [32]
NCH = 4              # LN chunks, 4 x 392
EPS = 1e-5
NCORES = 8
MASKB = -40.0        # additive mask bias (exp(s-40) ~ 1e-16)


def _row_stats(nc, pmm, prow, psq, ones, eps1, xs, n, sqdt):
    """Column mean / fused-LN scale for feature-major tiles xs (6 x [128,n]).

    Returns rows (r_mu, r_S, r_sd2) with S = rs1*rs2 the fused LN1+LN_att
    scale and sd2 = 1/rs2 (the y1-path scale)."""
    mu_ps = pmm.tile([1, n], F32, tag="mm", name="mu_ps")
    msq_ps = pmm.tile([1, n], F32, tag="mm", name="msq_ps")
    for dc in range(DC):
        nc.tensor.matmul(mu_ps[:], lhsT=ones[:], rhs=xs[dc],
                         start=(dc == 0), stop=(dc == DC - 1))
    for dc in range(DC):
        sq = psq.tile([128, n], sqdt, tag="sq")
        eng = nc.vector if dc % 3 != 2 else nc.gpsimd
        eng.tensor_mul(sq[:], xs[dc], xs[dc])
        nc.tensor.matmul(msq_ps[:], lhsT=ones[:], rhs=sq[:],
                         start=(dc == 0), stop=(dc == DC - 1))
    r_mu = prow.tile([1, n], F32, tag="row", name="r_mu")
    nc.vector.tensor_copy(r_mu[:], mu_ps[:])
    r_var = prow.tile([1, n], F32, tag="row", name="r_var")
    nc.vector.tensor_mul(r_var[:], r_mu[:], r_mu[:])
    nc.vector.tensor_sub(r_var[:], msq_ps[:], r_var[:])
    r_sd1 = prow.tile([1, n], F32, tag="row", name="r_sd1")
    nc.scalar.activation(r_sd1[:], r_var[:], AF.Sqrt, bias=eps1[0:1, 0:1])
    r_rs1 = prow.tile([1, n], F32, tag="row", name="r_rs1")
    nc.vector.reciprocal_approx_fast(r_rs1[:], r_sd1[:])
    r_v2 = prow.tile([1, n], F32, tag="row", name="r_v2")
    nc.vector.tensor_mul(r_v2[:], r_rs1[:], r_rs1[:])
    nc.vector.tensor_mul(r_v2[:], r_var[:], r_v2[:])      # var2 = var*rs1^2
    r_sd2 = prow.tile([1, n], F32, tag="row", name="r_sd2")
    nc.scalar.activation(r_sd2[:], r_v2[:], AF.Sqrt, bias=eps1[0:1, 0:1])
    r_S = prow.tile([1, n], F32, tag="row", name="r_S")
    nc.vector.reciprocal_approx_fast(r_S[:], r_sd2[:])
    nc.vector.tensor_mul(r_S[:], r_rs1[:], r_S[:])        # S = rs1*rs2
    return r_mu, r_S, r_sd2


def _bcast(nc, pbc, onesrow, row, n, name="bc"):
    """[1, n] SBUF fp32 row -> [128, n] PSUM via ones-matmul broadcast."""
    bc = pbc.tile([128, n], F32, tag="bc", name=name)
    nc.tensor.matmul(bc[:], lhsT=onesrow[:], rhs=row, start=True, stop=True)
    return bc


def _phase_ab(nc, tc, ctx, env):
    """LN1+LN_att fused normalization, then K/Q/V GEMMs into per-head tiles."""
    xp, wqkvP, mskr = env["xp"], env["wqkvP"], env["mskr"]
    ones, onesrow, eps1 = env["ones"], env["onesrow"], env["eps1"]
    KT, QT, VA, y1T = env["KT"], env["QT"], env["VA"], env["y1T"]
    bias_sb = env["bias_sb"]

    pxp = ctx.enter_context(tc.tile_pool(name="ab_x", bufs=3))
    pw = ctx.enter_context(tc.tile_pool(name="ab_w", bufs=1))
    pnt = ctx.enter_context(tc.tile_pool(name="ab_nt", bufs=DC))
    psq = ctx.enter_context(tc.tile_pool(name="ab_sq", bufs=3))
    ptmp = ctx.enter_context(tc.tile_pool(name="ab_tmp", bufs=2))
    prow = ctx.enter_context(tc.tile_pool(name="ab_rows", bufs=8))
    pstg = ctx.enter_context(tc.tile_pool(name="ab_stg", bufs=DC))
    pqstg = ctx.enter_context(tc.tile_pool(name="ab_qstg", bufs=1))

    # few large DMAs, interleaved so chunk-0 stats and K weights land first
    # (xc chunks 2/3 reuse chunk-0/1 buffers, so their DMAs go last in the
    # queue: they block on chunk-0/1 reads completing)
    wq = pw.tile([128, 3 * 4608], BF16, tag="wqkv")
    xc = [pxp.tile([128, DC * NQ], BF16, tag="xp", name=f"xp{ci}")
          for ci in range(NCH)]
    nc.sync.dma_start(xc[0][:], xp[:, 0:DC * NQ])
    nc.sync.dma_start(wq[:, 0:4608], wqkvP[:, 0:4608])              # K block
    nc.sync.dma_start(xc[1][:], xp[:, DC * NQ:2 * DC * NQ])
    nc.sync.dma_start(xc[2][:], xp[:, 2 * DC * NQ:3 * DC * NQ])
    nc.sync.dma_start(wq[:, 4608:9216], wqkvP[:, 4608:9216])        # Q block
    nc.sync.dma_start(wq[:, 9216:13824], wqkvP[:, 9216:13824])      # V block
    nc.sync.dma_start(xc[3][:], xp[:, 3 * DC * NQ:4 * DC * NQ])
    # mask/gate rows ride the idle GpSimd DGE queue so they never delay
    # the bulk input stream on the sync queue
    for h in range(H):
        nc.gpsimd.dma_start(KT[h][64:66, :], mskr[:])
        nc.gpsimd.dma_start(QT[h][64:66, :], env["qg"][:])

    nT = [pnt.tile([128, L], BF16, tag="nt", name=f"nT{i}") for i in range(DC)]
    stg = [pstg.tile([128, L], BF16, tag="kstg", name=f"kstg{i}")
           for i in range(DC)]

    with ExitStack() as ps1:
        pmm = ps1.enter_context(tc.tile_pool(name="ab_mm", bufs=4, space="PSUM"))
        pbc = ps1.enter_context(tc.tile_pool(name="ab_bc", bufs=2, space="PSUM"))
        pkps = ps1.enter_context(tc.tile_pool(name="ab_kps", bufs=2, space="PSUM"))

        def stats_mm(ci):
            xs = [xc[ci][:, dc * NQ:(dc + 1) * NQ] for dc in range(DC)]
            mu_ps = pmm.tile([1, NQ], F32, tag="mm", name="mu_ps")
            msq_ps = pmm.tile([1, NQ], F32, tag="mm", name="msq_ps")
            for dc in range(DC):
                nc.tensor.matmul(mu_ps[:], lhsT=ones[:], rhs=xs[dc],
                                 start=(dc == 0), stop=(dc == DC - 1))
            for dc in range(DC):
                sq = psq.tile([128, NQ], BF16, tag="sq")
                eng = nc.vector if dc % 3 != 2 else nc.gpsimd
                eng.tensor_mul(sq[:], xs[dc], xs[dc])
                nc.tensor.matmul(msq_ps[:], lhsT=ones[:], rhs=sq[:],
                                 start=(dc == 0), stop=(dc == DC - 1))
            return xs, mu_ps, msq_ps

        def finish_chunk(ci, xs, mu_ps, msq_ps):
            l0 = ci * NQ
            r_mu = prow.tile([1, NQ], F32, tag="row", name="r_mu")
            nc.vector.tensor_copy(r_mu[:], mu_ps[:])
            r_var = prow.tile([1, NQ], F32, tag="row", name="r_var")
            nc.vector.tensor_mul(r_var[:], r_mu[:], r_mu[:])
            nc.vector.tensor_sub(r_var[:], msq_ps[:], r_var[:])
            r_sd1 = prow.tile([1, NQ], F32, tag="row", name="r_sd1")
            nc.scalar.activation(r_sd1[:], r_var[:], AF.Sqrt, bias=eps1[0:1, 0:1])
            r_rs1 = prow.tile([1, NQ], F32, tag="row", name="r_rs1")
            nc.vector.reciprocal_approx_fast(r_rs1[:], r_sd1[:])
            r_v2 = prow.tile([1, NQ], F32, tag="row", name="r_v2")
            nc.vector.tensor_mul(r_v2[:], r_rs1[:], r_rs1[:])
            nc.vector.tensor_mul(r_v2[:], r_var[:], r_v2[:])
            r_sd2 = prow.tile([1, NQ], F32, tag="row", name="r_sd2")
            nc.scalar.activation(r_sd2[:], r_v2[:], AF.Sqrt, bias=eps1[0:1, 0:1])
            r_S = prow.tile([1, NQ], F32, tag="row", name="r_S")
            nc.vector.reciprocal_approx_fast(r_S[:], r_sd2[:])
            nc.vector.tensor_mul(r_S[:], r_rs1[:], r_S[:])
            mu_b = _bcast(nc, pbc, onesrow, r_mu[:], NQ, "mu_b")
            S_b = _bcast(nc, pbc, onesrow, r_S[:], NQ, "S_b")
            for dc in range(DC):
                tmp = ptmp.tile([128, NQ], F32, tag="tmpa")
                nc.vector.tensor_sub(tmp[:], xs[dc], mu_b[:])
                nc.vector.tensor_mul(nT[dc][:, l0:l0 + NQ], tmp[:], S_b[:])
            if ci == NCH - 1:
                y_b = _bcast(nc, pbc, onesrow, r_sd2[:], NQ, "y_b")
                for dc in range(DC):
                    nc.vector.tensor_mul(y1T[dc][:], nT[dc][:, Q0:L], y_b[:])
            for et in range(DC):
                ps = pkps.tile([128, NQ], F32, tag="kps")
                for dc in range(DC):
                    nc.tensor.matmul(
                        ps[:], lhsT=wq[:, dc * 768 + et * 128:dc * 768 + (et + 1) * 128],
                        rhs=nT[dc][:, l0:l0 + NQ],
                        start=(dc == 0), stop=(dc == DC - 1))
                nc.scalar.copy(KT[2 * et][0:64, l0:l0 + NQ], ps[0:64, :])
                nc.vector.tensor_copy(stg[et][64:128, l0:l0 + NQ], ps[64:128, :])

        pend = None   # stats for chunk ci+1 issue before chunk ci's rows/
        for ci in range(NCH):   # nT/K, keeping the PE fed during row math
            cur = stats_mm(ci)
            if pend is not None:
                finish_chunk(ci - 1, *pend)
            pend = cur
        finish_chunk(NCH - 1, *pend)
    for et in range(DC):
        nc.gpsimd.dma_start(KT[2 * et + 1][0:64, :], stg[et][64:128, :])

    with ExitStack() as ps2:
        pkps = ps2.enter_context(tc.tile_pool(name="ab_kps", bufs=2, space="PSUM"))
        pqps = ps2.enter_context(tc.tile_pool(name="ab_qps", bufs=2, space="PSUM"))
        pvps = ps2.enter_context(tc.tile_pool(name="ab_vps", bufs=2, space="PSUM"))
        # K^T: even head lands in its [66, L] tile directly; odd head is
        # staged (engines cannot shift partitions) and DMA'd to partition 0.
        for et in range(DC):
            stg = pstg.tile([128, L], BF16, tag="kstg")
            for ci in range(NCH):
                l0 = ci * NQ
                ps = pkps.tile([128, NQ], F32, tag="kps")
                for dc in range(DC):
                    nc.tensor.matmul(
                        ps[:], lhsT=wq[:, dc * 768 + et * 128:dc * 768 + (et + 1) * 128],
                        rhs=nT[dc][:, l0:l0 + NQ],
                        start=(dc == 0), stop=(dc == DC - 1))
                nc.scalar.copy(KT[2 * et][0:64, l0:l0 + NQ], ps[0:64, :])
                nc.vector.tensor_copy(stg[64:128, l0:l0 + NQ], ps[64:128, :])
            nc.sync.dma_start(KT[2 * et + 1][0:64, :], stg[64:128, :])
        # Q^T with folded bias -> per-head [66, 392] tiles (rows 64/65 gates)
        qstg = pqstg.tile([128, DC * NQ], BF16, tag="qstg")
        for et in range(DC):
            ps = pqps.tile([128, NQ], F32, tag="qps")
            for dc in range(DC):
                nc.tensor.matmul(
                    ps[:], lhsT=wq[:, 4608 + dc * 768 + et * 128:4608 + dc * 768 + (et + 1) * 128],
                    rhs=nT[dc][:, Q0:L], start=(dc == 0), stop=(dc == DC - 1))
            nc.scalar.activation(QT[2 * et][0:64, :], ps[0:64, :], AF.Identity,
                                 bias=bias_sb[0:64, et:et + 1])
            nc.scalar.activation(qstg[64:128, et * NQ:(et + 1) * NQ], ps[64:128, :],
                                 AF.Identity, bias=bias_sb[64:128, et:et + 1])
        for et in range(DC):
            nc.gpsimd.dma_start(QT[2 * et + 1][0:64, :],
                              qstg[64:128, et * NQ:(et + 1) * NQ])

        # V token-major with a ones column per head (softmax denominators)
        for lt in range(NJT):
            lsz = JSZ[lt]
            ps_v = pvps.tile([128, D], F32, tag="psv")
            for dc in range(DC):
                lhsT = nT[dc][:, lt * 128:lt * 128 + lsz]
                rhs = wq[:, 9216 + dc * 768:9216 + (dc + 1) * 768]
                nc.tensor.matmul(ps_v[0:lsz, 0:512], lhsT=lhsT, rhs=rhs[:, 0:512],
                                 start=(dc == 0), stop=(dc == DC - 1),
                                 skip_group_check=True)
                nc.tensor.matmul(ps_v[0:lsz, 512:D], lhsT=lhsT, rhs=rhs[:, 512:D],
                                 start=(dc == 0), stop=(dc == DC - 1),
                                 skip_group_check=True)
            vav = VA[lt][:].rearrange("p (h c) -> p h c", c=65)
            eng = nc.vector if lt % 2 == 0 else nc.gpsimd
            vcv = env["vcolb"][:].rearrange("p (h o) -> p h o", o=1)
            eng.tensor_copy(vav[:, :, 64:65], vcv)
            nc.scalar.copy(vav[0:lsz, :, 0:64],
                           ps_v[0:lsz, :].rearrange("p (h c) -> p h c", c=64))


def _phase_attn(nc, tc, ctx, env):
    """Mask-fused scores (contraction 66), paired exp, PV with ones column.
    Each head is normalized as soon as its denominator lands (fast DVE
    reciprocal, PE ones-broadcast) so the out-projection afterwards is one
    dense PE burst."""
    KT, QT, VA = env["KT"], env["QT"], env["VA"]
    y1T, x2T, bias_sb = env["y1T"], env["x2T"], env["bias_sb"]
    onesrow64 = env["onesrow64"]

    pwD = ctx.enter_context(tc.tile_pool(name="d_w", bufs=1))
    wo = pwD.tile([64, H * D], BF16, tag="wD")
    nc.sync.dma_start(wo[:], env["woutP"][:])
    w1 = env["pw12"].tile([128, DC * DFF], BF16, tag="w1")
    nc.sync.dma_start(w1[:], env["w1P"][:])
    env["w1"] = w1
    w2 = env["pw12"].tile([128, FT * D], BF16, tag="w2")
    nc.sync.dma_start(w2[:], env["w2P"][:])
    env["w2"] = w2

    pos = ctx.enter_context(tc.tile_pool(name="c_os", bufs=2))
    pont = ctx.enter_context(tc.tile_pool(name="c_ontp", bufs=H))
    prn = ctx.enter_context(tc.tile_pool(name="c_rn", bufs=2))
    ONT = [pont.tile([64, NQ], BF16, tag="ont", name=f"ONT{h}")
           for h in range(H)]

    with ExitStack() as hctx:
        ppt = hctx.enter_context(tc.tile_pool(name="c_pt", bufs=3))
        pss = hctx.enter_context(tc.tile_pool(name="c_ps_s", bufs=2, space="PSUM"))
        pso = hctx.enter_context(tc.tile_pool(name="c_ps_o", bufs=2, space="PSUM"))
        pbcH = hctx.enter_context(tc.tile_pool(name="c_bch", bufs=2, space="PSUM"))

        def emit_pv(h, o_ps, jts, pt):
            for k, jt in enumerate(jts):
                nc.tensor.matmul(
                    o_ps[:], lhsT=VA[jt][0:JSZ[jt], h * 65:(h + 1) * 65],
                    rhs=pt[0:JSZ[jt], k * NQ:k * NQ + NQ],
                    start=(jt == 0), stop=(jt == NJT - 1),
                    skip_group_check=True)
            if jts[-1] == NJT - 1:
                # head complete: stage out of PSUM and normalize right away
                oS = pos.tile([65, NQ], BF16, tag="os", name=f"oS{h % 2}")
                nc.vector.tensor_copy(oS[:], o_ps[:])
                dh = prn.tile([1, NQ], BF16, tag="dh")
                nc.gpsimd.dma_start(dh[:], oS[64:65, :])
                dhf = prn.tile([1, NQ], F32, tag="dhf")
                nc.vector.tensor_copy(dhf[:], dh[:])
                dr = prn.tile([1, NQ], F32, tag="dr")
                nc.vector.reciprocal_approx_fast(dr[:], dhf[:])
                drb = prn.tile([1, NQ], BF16, tag="drb")
                nc.vector.tensor_copy(drb[:], dr[:])
                bc = pbcH.tile([64, NQ], F32, tag="rb")
                nc.tensor.matmul(bc[:], lhsT=onesrow64[:], rhs=drb[:],
                                 start=True, stop=True)
                nc.vector.tensor_mul(ONT[h][:], oS[0:64, :], bc[:])

        pend = None    # 1-deep pipeline ACROSS heads: scores of the next
        for h in range(H):  # pair issue before PV of the current one
            o_ps = pso.tile([65, NQ], F32, tag="pso", name=f"o_ps{h % 2}")
            for jp in range(7):
                jts = [jt for jt in (2 * jp, 2 * jp + 1) if jt < NJT]
                s2 = pss.tile([128, 1024], F32, tag="ps_s")
                for k, jt in enumerate(jts):
                    nc.tensor.matmul(
                        s2[0:JSZ[jt], k * 512:k * 512 + NQ],
                        lhsT=KT[h][0:66, jt * 128:jt * 128 + JSZ[jt]],
                        rhs=QT[h][0:66, :], start=True, stop=True,
                        skip_group_check=True)
                pt = ppt.tile([128, 2 * NQ], BF16, tag="pt")
                if len(jts) == 2:
                    s2v = s2[:].rearrange("p (s c) -> p s c", c=512)[:, :, 0:NQ]
                    ptv = pt[:].rearrange("p (s c) -> p s c", c=NQ)
                    nc.scalar.activation(ptv[:], s2v, AF.Exp)
                else:
                    nc.scalar.activation(pt[:, 0:NQ], s2[:, 0:NQ], AF.Exp)
                if pend is not None:
                    emit_pv(*pend)
                pend = (h, o_ps, jts, pt)
        emit_pv(*pend)

    # out-projection: one dense accumulation burst over all heads
    with ExitStack() as nctx:
        pop = nctx.enter_context(tc.tile_pool(name="c_op", bufs=DC, space="PSUM"))
        ps_op = [pop.tile([128, NQ], F32, tag="op", name=f"ps_op{i}")
                 for i in range(DC)]
        for h in range(H):
            for dt in range(DC):
                nc.tensor.matmul(
                    ps_op[dt][:], lhsT=wo[0:64, h * D + dt * 128:h * D + (dt + 1) * 128],
                    rhs=ONT[h][:], start=(h == 0), stop=(h == H - 1),
                    skip_group_check=True)
        for dt in range(DC):
            nc.vector.scalar_tensor_tensor(
                x2T[dt][:], ps_op[dt][:], bias_sb[:, 6 + dt:7 + dt], y1T[dt][:],
                op0=ALU.add, op1=ALU.add)


def _phase_ffn(nc, tc, ctx, env):
    """LN2 + interleaved FFN1(silu)/FFN2 with residual."""
    onesD32, onesrow, eps1 = env["onesD32"], env["onesrow"], env["eps1"]
    bias_sb = env["bias_sb"]
    x2T, w1, w2 = env["x2T"], env["w1"], env["w2"]

    psq2 = ctx.enter_context(tc.tile_pool(name="e_sq", bufs=3))
    ptmp2 = ctx.enter_context(tc.tile_pool(name="e_tmp", bufs=3))
    prow3 = ctx.enter_context(tc.tile_pool(name="e_rows", bufs=8))
    pn2 = ctx.enter_context(tc.tile_pool(name="e_n2", bufs=DC))
    pffs = ctx.enter_context(tc.tile_pool(name="f_ffs", bufs=2))
    poutT = ctx.enter_context(tc.tile_pool(name="f_outT", bufs=1))

    n2T = [pn2.tile([128, NQ], BF16, tag="n2", name=f"n2T{i}") for i in range(DC)]
    with ExitStack() as lctx:
        pmm2 = lctx.enter_context(tc.tile_pool(name="e_mm", bufs=3, space="PSUM"))
        pbc3 = lctx.enter_context(tc.tile_pool(name="e_bc", bufs=2, space="PSUM"))
        xs = [x2T[dc][:] for dc in range(DC)]
        r_mu2, r_S2, _ = _row_stats(nc, pmm2, prow3, psq2, onesD32, eps1,
                                    xs, NQ, F32R)
        mu2_b = _bcast(nc, pbc3, onesrow, r_mu2[:], NQ, "mu2_b")
        S2_b = _bcast(nc, pbc3, onesrow, r_S2[:], NQ, "S2_b")
        for dc in range(DC):
            tmp = ptmp2.tile([128, NQ], F32, tag="tmp2")
            nc.vector.tensor_sub(tmp[:], x2T[dc][:], mu2_b[:])
            nc.vector.tensor_mul(n2T[dc][:], tmp[:], S2_b[:])

    outT = poutT.tile([128, DC * NQ], F32, tag="outT")
    with ExitStack() as fctx:
        pmmE = fctx.enter_context(tc.tile_pool(name="f_mm", bufs=2, space="PSUM"))
        pacc = fctx.enter_context(tc.tile_pool(name="f_acc", bufs=DC, space="PSUM"))
        ps_acc = [pacc.tile([128, NQ], F32, tag="acc", name=f"ps_acc{i}")
                  for i in range(DC)]
        for ft in range(FT):
            ps1 = pmmE.tile([128, NQ], F32, tag="mm", name="ps1")
            for dc in range(DC):
                nc.tensor.matmul(
                    ps1[:], lhsT=w1[:, dc * DFF + ft * 128:dc * DFF + (ft + 1) * 128],
                    rhs=n2T[dc][:], start=(dc == 0), stop=(dc == DC - 1))
            # silu(u) = u * sigmoid(u) with u = ps1 + cb1
            sig = pffs.tile([128, NQ], BF16, tag="sig")
            nc.scalar.activation(sig[:], ps1[:], AF.Sigmoid,
                                 bias=bias_sb[:, 18 + ft:19 + ft])
            ffs = pffs.tile([128, NQ], BF16, tag="ffs")
            nc.vector.scalar_tensor_tensor(ffs[:], ps1[:], bias_sb[:, 18 + ft:19 + ft],
                                           sig[:], op0=ALU.add, op1=ALU.mult)
            for dt in range(DC):
                nc.tensor.matmul(
                    ps_acc[dt][:], lhsT=w2[:, ft * D + dt * 128:ft * D + (dt + 1) * 128],
                    rhs=ffs[:], start=(ft == 0), stop=(ft == FT - 1),
                    skip_group_check=True)
        for dt in range(DC):
            nc.vector.scalar_tensor_tensor(
                outT[:, dt * NQ:(dt + 1) * NQ], ps_acc[dt][:],
                bias_sb[:, 12 + dt:13 + dt], x2T[dt][:],
                op0=ALU.add, op1=ALU.add)
    nc.sync.dma_start(env["out"][:], outT[:])


def build_program():
    nc = bacc.Bacc("TRN2")
    env = {}
    env["xp"] = nc.declare_dram_parameter("xp", [128, NCH * DC * NQ], BF16, isOutput=False)
    env["wqkvP"] = nc.declare_dram_parameter("wqkvP", [128, 3 * 4608], BF16, isOutput=False)
    env["mskr"] = nc.declare_dram_parameter("mskr", [2, L], BF16, isOutput=False)
    env["qg"] = nc.declare_dram_parameter("qg", [2, NQ], BF16, isOutput=False)
    env["woutP"] = nc.declare_dram_parameter("woutP", [64, H * D], BF16, isOutput=False)
    env["w1P"] = nc.declare_dram_parameter("w1P", [128, DC * DFF], BF16, isOutput=False)
    env["w2P"] = nc.declare_dram_parameter("w2P", [128, FT * D], BF16, isOutput=False)
    biasP = nc.declare_dram_parameter("biasP", [128, 42], F32, isOutput=False)
    env["out"] = nc.declare_dram_parameter("out", [128, DC * NQ], F32, isOutput=True)

    with tile.TileContext(nc) as tc, ExitStack() as top:
        pc = top.enter_context(tc.tile_pool(name="const", bufs=1))
        px2 = top.enter_context(tc.tile_pool(name="x2p", bufs=DC))

        onesf = pc.tile([128, 1], F32, tag="onesf")
        nc.vector.memset(onesf[:], 1.0 / D)
        ones = pc.tile([128, 1], BF16, tag="ones")
        nc.vector.tensor_copy(ones[:], onesf[:])
        onesD32 = pc.tile([128, 1], F32R, tag="ones32")
        nc.vector.tensor_copy(onesD32[:], onesf[:])
        onesrow = pc.tile([1, 128], F32, tag="onesrow")
        nc.vector.memset(onesrow[:], 1.0)
        onesrow64 = pc.tile([1, 64], BF16, tag="onesrow64")
        nc.vector.tensor_copy(onesrow64[:], onesrow[0:1, 0:64])
        eps1 = pc.tile([1, 1], F32, tag="eps1")
        nc.vector.memset(eps1[:], EPS)
        vcolf = pc.tile([128, H], F32, tag="vcolf")
        nc.vector.memset(vcolf[:], 1.0)
        vcolb = pc.tile([128, H], BF16, tag="vcolb")
        nc.vector.tensor_copy(vcolb[:], vcolf[:])
        env["vcolb"] = vcolb
        bias_sb = pc.tile([128, 42], F32, tag="bias")
        nc.sync.dma_start(bias_sb[:], biasP[:])
        env.update(ones=ones, onesD32=onesD32, onesrow=onesrow,
                   onesrow64=onesrow64, eps1=eps1, bias_sb=bias_sb)

        env["x2T"] = [px2.tile([128, NQ], F32R, tag="x2", name=f"x2T{i}")
                      for i in range(DC)]

        with ExitStack() as mid:
            pkt = mid.enter_context(tc.tile_pool(name="ktp", bufs=H))
            pqt = mid.enter_context(tc.tile_pool(name="qtp", bufs=H))
            pva = mid.enter_context(tc.tile_pool(name="vap", bufs=NJT))
            py1 = mid.enter_context(tc.tile_pool(name="y1p", bufs=DC))
            env["KT"] = [pkt.tile([66, L], BF16, tag="kt", name=f"KT{i}")
                         for i in range(H)]
            env["QT"] = [pqt.tile([66, NQ], BF16, tag="qt", name=f"QT{i}")
                         for i in range(H)]
            env["VA"] = [pva.tile([128, H * 65], BF16, tag="va", name=f"VA{i}")
                         for i in range(NJT)]
            env["y1T"] = [py1.tile([128, NQ], BF16, tag="y1", name=f"y1T{i}")
                          for i in range(DC)]

            with ExitStack() as ctx:
                _phase_ab(nc, tc, ctx, env)

            # FFN weight pool opens only now: during phase AB its 72KB would
            # starve SBUF, and the FFN (also under `mid`) still sees it.
            env["pw12"] = mid.enter_context(tc.tile_pool(name="w12p", bufs=1))

            with ExitStack() as ctx:
                _phase_attn(nc, tc, ctx, env)

            with ExitStack() as ctx:
                _phase_ffn(nc, tc, ctx, env)

    nc.finalize()
    return nc


_NC = None


def _get_nc():
    global _NC
    if _NC is None:
        _NC = build_program()
    return _NC


def _host_prepare(inputs):
    """Fold constants and lay out per-core input maps (pure layout work)."""
    f32 = np.float32
    x = np.asarray(inputs["x"], f32)
    memory = np.asarray(inputs["memory"], f32)
    w_qkv = np.asarray(inputs["w_qkv"], f32)
    w_out = np.asarray(inputs["w_out"], f32)
    b_out = np.asarray(inputs["b_out"], f32)
    g_att = np.asarray(inputs["ln_att_g"], f32)
    b_att = np.asarray(inputs["ln_att_b"], f32)
    g2 = np.asarray(inputs["ln2_g"], f32)
    bb2 = np.asarray(inputs["ln2_b"], f32)
    w1 = np.asarray(inputs["w1"], f32)
    b1 = np.asarray(inputs["b1"], f32)
    w2 = np.asarray(inputs["w2"], f32)
    b2v = np.asarray(inputs["b2"], f32)

    qscale = f32(64 ** -0.5)
    w_qkv_eff = w_qkv * g_att[None, :]
    w_qkv_eff[:D] *= qscale
    cb_qkv = w_qkv @ b_att
    cb_q = (cb_qkv[:D] * qscale).astype(f32)
    cb_v = cb_qkv[2 * D:].astype(f32)
    b_out_eff = (b_out + w_out @ cb_v).astype(f32)
    w1_eff = w1 * g2[None, :]
    cb1_eff = (w1 @ bb2 + b1).astype(f32)

    def cols(v):
        return np.ascontiguousarray(v.reshape(-1, 128).T)

    biasP = np.zeros((128, 42), f32)
    biasP[:, 0:6] = cols(cb_q)
    biasP[:, 6:12] = cols(b_out_eff)
    biasP[:, 12:18] = cols(b2v)
    biasP[:, 18:42] = cols(cb1_eff)

    def packP(wT, ncol):
        # [D_in, ncol] -> [128, (D_in/128)*ncol] partition-packed bf16
        return np.ascontiguousarray(
            wT.reshape(-1, 128, ncol).transpose(1, 0, 2).reshape(128, -1)
        ).astype(NPBF16)

    wq_T = np.ascontiguousarray(w_qkv_eff.T)       # [D, 3D]
    wqkvP = np.concatenate(
        [packP(np.ascontiguousarray(wq_T[:, D:2 * D]), D),      # K
         packP(np.ascontiguousarray(wq_T[:, 0:D]), D),          # Q
         packP(np.ascontiguousarray(wq_T[:, 2 * D:3 * D]), D)], # V
        axis=1)
    woutP = np.ascontiguousarray(
        w_out.T.reshape(H, 64, D).transpose(1, 0, 2).reshape(64, H * D)
    ).astype(NPBF16)

    shared = {
        "wqkvP": wqkvP,
        "woutP": woutP,
        "w1P": packP(np.ascontiguousarray(w1_eff.T), DFF),
        "w2P": packP(np.ascontiguousarray(w2.T), D),
        "biasP": biasP,
    }

    perm0 = np.concatenate([np.arange(0, T), np.arange(Q0, L), np.arange(T, Q0)])
    in_maps = []
    for c in range(NCORES):
        b, hf = divmod(c, 2)
        x_aug = np.concatenate([memory[b, :T], x[b]], axis=0)      # [L, D]
        old = perm0 if hf == 0 else np.arange(L)
        xa = x_aug[old]
        # [p, ci*6*392 + dc*392 + q] = xa[ci*392+q, dc*128+p]
        xp = np.ascontiguousarray(
            xa.T.reshape(DC, 128, NCH, NQ).transpose(1, 2, 0, 3).reshape(128, -1)
        ).astype(NPBF16)
        LcA = (5 + 2 * hf) * NPATCH
        LcB = (6 + 2 * hf) * NPATCH
        mb = np.where(old < LcB, 0.0, MASKB).astype(f32)
        ma = np.where(old < LcA, 0.0, MASKB).astype(f32)
        mskr = np.stack([mb, ma - mb]).astype(NPBF16)
        qg = np.stack([np.ones(NQ, f32),
                       (np.arange(NQ) < NPATCH).astype(f32)]).astype(NPBF16)
        in_maps.append({"xp": xp, "mskr": mskr, "qg": qg, **shared})
    return in_maps


def _assemble(results):
    out = np.zeros((B, T, D), np.float32)
    for c in range(NCORES):
        b, hf = divmod(c, 2)
        fm = results[c]["out"].reshape(128, DC, NQ).transpose(1, 0, 2).reshape(D, NQ)
        out[b, hf * NQ:(hf + 1) * NQ, :] = fm.T
    return out


def kernel(**inputs):
    nc = _get_nc()
    in_maps = _host_prepare(inputs)
    res = run_bass_kernel_spmd(nc, in_maps, list(range(NCORES)))
    return _assemble(res.results)


def _ensure_ntff_hook():
    """Provide antenv.axon_hooks (absent in this image) so trace=True can
    drive NTFF capture through libaxon_pjrt.so, mirroring trn_boot.py."""
    import contextlib
    import ctypes
    import types

    try:
        from antenv.axon_hooks import get_axon_ntff_profile_hook  # noqa: F401
        return
    except ImportError:
        pass
    import antenv

    so_path = "/opt/axon/libaxon_pjrt.so"
    lib = ctypes.CDLL(so_path)
    if not hasattr(lib, "axon_start_nrt_profile"):
        raise RuntimeError("libaxon_pjrt.so lacks NTFF profile symbols")
    lib.axon_start_nrt_profile.argtypes = [ctypes.POINTER(ctypes.c_int64),
                                           ctypes.c_size_t]
    lib.axon_start_nrt_profile.restype = ctypes.c_int64
    lib.axon_stop_nrt_profile.argtypes = [ctypes.c_char_p]
    lib.axon_stop_nrt_profile.restype = ctypes.c_int64

    @contextlib.contextmanager
    def _hook(output_dir, device_ids):
        import jax
        jax.devices()
        if device_ids:
            ids = (ctypes.c_int64 * len(device_ids))(*device_ids)
            rc = lib.axon_start_nrt_profile(ids, len(device_ids))
        else:
            rc = lib.axon_start_nrt_profile(None, 0)
        if rc != 0:
            raise RuntimeError(f"axon_start_nrt_profile rc={rc}")
        try:
            yield
        finally:
            n = lib.axon_stop_nrt_profile(str(output_dir).encode())
            print(f"ntff profile: {n} file(s) written to {output_dir}",
                  file=sys.stderr)

    box = {"h": _hook}
    mod = types.ModuleType("antenv.axon_hooks")
    mod.set_axon_ntff_profile_hook = lambda h: box.__setitem__("h", h)
    mod.get_axon_ntff_profile_hook = lambda: box["h"]
    sys.modules["antenv.axon_hooks"] = mod
    antenv.axon_hooks = mod


def kernel_traced(**inputs):
    """Like kernel() but with NTFF profiling; returns (out, exec_time_ns)."""
    import tempfile

    from concourse import bass_utils as _bu
    _ensure_ntff_hook()
    _bu.upload_artifacts = lambda tmpdir: f"local:{tmpdir}"  # no bucket creds here
    nc = _get_nc()
    in_maps = _host_prepare(inputs)
    tmpdir = tempfile.mkdtemp(prefix="ntff_")
    res = run_bass_kernel_spmd(nc, in_maps, list(range(NCORES)), trace=True,
                               tmpdir=tmpdir)
    return _assemble(res.results), res.exec_time_ns


# revision 26
# speedup vs baseline: 1.0613x; 1.0613x over previous
"""Trainium2 Bass kernel: LookupTransformerBlock (block-causal sparse attention).

Reference semantics (B=4, T=784, D=768, H=12, Dh=64, d_ff=3072):
  x_aug = LN1(concat(memory[:, :T], x))              # [B, 2T, D]
  h     = LN_att(x_aug)
  qkv   = h @ w_qkv.T ; block-causal attention over frames of 196
  x2    = x_aug + attn_out
  out   = (x2 + FFN(LN2(x2)))[:, T:, :]

Sharding: 8 cores = (batch b in 0..3) x (query-half hf in 0..1); each core
computes its 392 output rows with K/V over all 1568 positions (data-parallel,
no collectives).  One SPMD program; per-core differences live in input data.

Layout decisions (driven by the DMA/engine-overhead analysis of the previous
version's trace — 500+ small DMAs and per-tile mask exps were the bottleneck):
  - every DRAM tensor is packed host-side so a handful of dma_starts move
    everything with multi-KB per-partition contiguous rows, weights and
    activations in bf16;
  - the host permutes key columns per core so the query slice is always
    columns [1176:1568) -> LN1 chunk-3 stats double as the query stats and
    q-tiles are plain slices;
  - the attention mask is folded into the score matmul: per-head K tiles
    carry two extra contraction rows (frame-B mask bias, frame-A correction)
    and per-head Q tiles the matching gate rows (1, 1_{query in frame A}).
    Exp is then mask-free, so score j-tiles are exp'd in pairs out of PSUM;
  - softmax denominators come from a ones column in V, leave PSUM through
    per-head row DMAs, hit one batched reciprocal_approx_fast, and return
    as PE ones-matmul broadcasts (no DRAM bounces anywhere);
  - the final output stays feature-major on device; the host transposes.
"""

import os
import sys
from contextlib import ExitStack

import numpy as np
import ml_dtypes

for _p in ("/opt/trn_rl_repo", os.path.expanduser("~/.axon_site/_ro/trn_rl_repo")):
    if os.path.isdir(_p) and _p not in sys.path:
        sys.path.append(_p)

import concourse.bass as bass
import concourse.bacc as bacc
import concourse.mybir as mybir
import concourse.tile as tile
from concourse.bass_utils import run_bass_kernel_spmd

F32 = mybir.dt.float32
F32R = mybir.dt.float32r
BF16 = mybir.dt.bfloat16
AF = mybir.ActivationFunctionType
ALU = mybir.AluOpType
NPBF16 = ml_dtypes.bfloat16

B = 4
T = 784
D = 768
L = 2 * T            # 1568
NQ = 392             # query rows per core (= LN chunk width)
Q0 = L - NQ          # queries always live at columns [1176:1568)
H = 12
DFF = 3072
NPATCH = 196
DC = D // 128        # 6
FT = DFF // 128      # 24
NJT = 13             # j-tiles over L (12 x 128 + 32)
JSZ = [128] * 12 + [32]
NCH = 4              # LN chunks, 4 x 392
EPS = 1e-5
NCORES = 8
MASKB = -40.0        # additive mask bias (exp(s-40) ~ 1e-16)


def _row_stats(nc, pmm, prow, psq, ones, eps1, xs, n, sqdt):
    """Column mean / fused-LN scale for feature-major tiles xs (6 x [128,n]).

    Returns rows (r_mu, r_S, r_sd2) with S = rs1*rs2 the fused LN1+LN_att
    scale and sd2 = 1/rs2 (the y1-path scale)."""
    mu_ps = pmm.tile([1, n], F32, tag="mm", name="mu_ps")
    msq_ps = pmm.tile([1, n], F32, tag="mm", name="msq_ps")
    for dc in range(DC):
        nc.tensor.matmul(mu_ps[:], lhsT=ones[:], rhs=xs[dc],
                         start=(dc == 0), stop=(dc == DC - 1))
    for dc in range(DC):
        sq = psq.tile([128, n], sqdt, tag="sq")
        eng = nc.vector if dc % 3 != 2 else nc.gpsimd
        eng.tensor_mul(sq[:], xs[dc], xs[dc])
        nc.tensor.matmul(msq_ps[:], lhsT=ones[:], rhs=sq[:],
                         start=(dc == 0), stop=(dc == DC - 1))
    r_mu = prow.tile([1, n], F32, tag="row", name="r_mu")
    nc.vector.tensor_copy(r_mu[:], mu_ps[:])
    r_var = prow.tile([1, n], F32, tag="row", name="r_var")
    nc.vector.tensor_mul(r_var[:], r_mu[:], r_mu[:])
    nc.vector.tensor_sub(r_var[:], msq_ps[:], r_var[:])
    r_sd1 = prow.tile([1, n], F32, tag="row", name="r_sd1")
    nc.scalar.activation(r_sd1[:], r_var[:], AF.Sqrt, bias=eps1[0:1, 0:1])
    r_rs1 = prow.tile([1, n], F32, tag="row", name="r_rs1")
    nc.vector.reciprocal_approx_fast(r_rs1[:], r_sd1[:])
    r_v2 = prow.tile([1, n], F32, tag="row", name="r_v2")
    nc.vector.tensor_mul(r_v2[:], r_rs1[:], r_rs1[:])
    nc.vector.tensor_mul(r_v2[:], r_var[:], r_v2[:])      # var2 = var*rs1^2
    r_sd2 = prow.tile([1, n], F32, tag="row", name="r_sd2")
    nc.scalar.activation(r_sd2[:], r_v2[:], AF.Sqrt, bias=eps1[0:1, 0:1])
    r_S = prow.tile([1, n], F32, tag="row", name="r_S")
    nc.vector.reciprocal_approx_fast(r_S[:], r_sd2[:])
    nc.vector.tensor_mul(r_S[:], r_rs1[:], r_S[:])        # S = rs1*rs2
    return r_mu, r_S, r_sd2


def _bcast(nc, pbc, onesrow, row, n, name="bc"):
    """[1, n] SBUF fp32 row -> [128, n] PSUM via ones-matmul broadcast."""
    bc = pbc.tile([128, n], F32, tag="bc", name=name)
    nc.tensor.matmul(bc[:], lhsT=onesrow[:], rhs=row, start=True, stop=True)
    return bc


def _phase_ab(nc, tc, ctx, env):
    """LN1+LN_att fused normalization, then K/Q/V GEMMs into per-head tiles."""
    xp, wqkvP, mskr = env["xp"], env["wqkvP"], env["mskr"]
    ones, onesrow, eps1 = env["ones"], env["onesrow"], env["eps1"]
    KT, QT, VA, y1T = env["KT"], env["QT"], env["VA"], env["y1T"]
    bias_sb = env["bias_sb"]

    pxp = ctx.enter_context(tc.tile_pool(name="ab_x", bufs=3))
    pw = ctx.enter_context(tc.tile_pool(name="ab_w", bufs=1))
    pnt = ctx.enter_context(tc.tile_pool(name="ab_nt", bufs=DC))
    psq = ctx.enter_context(tc.tile_pool(name="ab_sq", bufs=2))
    ptmp = ctx.enter_context(tc.tile_pool(name="ab_tmp", bufs=2))
    prow = ctx.enter_context(tc.tile_pool(name="ab_rows", bufs=7))
    pstg = ctx.enter_context(tc.tile_pool(name="ab_stg", bufs=DC))
    pqstg = ctx.enter_context(tc.tile_pool(name="ab_qstg", bufs=1))

    # few large DMAs, interleaved so chunk-0 stats and K weights land first
    # (xc chunks 2/3 reuse chunk-0/1 buffers, so their DMAs go last in the
    # queue: they block on chunk-0/1 reads completing)
    wq = pw.tile([128, 3 * 4608], BF16, tag="wqkv")
    xc = [pxp.tile([128, DC * NQ], BF16, tag="xp", name=f"xp{ci}")
          for ci in range(NCH)]
    nc.sync.dma_start(xc[0][:], xp[:, 0:DC * NQ])
    nc.sync.dma_start(wq[:, 0:4608], wqkvP[:, 0:4608])              # K block
    nc.sync.dma_start(xc[1][:], xp[:, DC * NQ:2 * DC * NQ])
    nc.sync.dma_start(xc[2][:], xp[:, 2 * DC * NQ:3 * DC * NQ])
    nc.sync.dma_start(wq[:, 4608:9216], wqkvP[:, 4608:9216])        # Q block
    nc.sync.dma_start(wq[:, 9216:13824], wqkvP[:, 9216:13824])      # V block
    nc.sync.dma_start(xc[3][:], xp[:, 3 * DC * NQ:4 * DC * NQ])
    # mask/gate rows ride the idle GpSimd DGE queue so they never delay
    # the bulk input stream on the sync queue
    for h in range(H):
        nc.gpsimd.dma_start(KT[h][64:66, :], mskr[:])
        nc.gpsimd.dma_start(QT[h][64:66, :], env["qg"][:])

    nT = [pnt.tile([128, L], BF16, tag="nt", name=f"nT{i}") for i in range(DC)]
    stg = [pstg.tile([128, L], BF16, tag="kstg", name=f"kstg{i}")
           for i in range(DC)]

    with ExitStack() as ps1:
        pmm = ps1.enter_context(tc.tile_pool(name="ab_mm", bufs=4, space="PSUM"))
        pbc = ps1.enter_context(tc.tile_pool(name="ab_bc", bufs=2, space="PSUM"))
        pkps = ps1.enter_context(tc.tile_pool(name="ab_kps", bufs=2, space="PSUM"))

        def stats_mm(ci):
            xs = [xc[ci][:, dc * NQ:(dc + 1) * NQ] for dc in range(DC)]
            mu_ps = pmm.tile([1, NQ], F32, tag="mm", name="mu_ps")
            msq_ps = pmm.tile([1, NQ], F32, tag="mm", name="msq_ps")
            for dc in range(DC):
                nc.tensor.matmul(mu_ps[:], lhsT=ones[:], rhs=xs[dc],
                                 start=(dc == 0), stop=(dc == DC - 1))
            for dc in range(DC):
                sq = psq.tile([128, NQ], BF16, tag="sq")
                eng = nc.vector if dc % 3 != 2 else nc.gpsimd
                eng.tensor_mul(sq[:], xs[dc], xs[dc])
                nc.tensor.matmul(msq_ps[:], lhsT=ones[:], rhs=sq[:],
                                 start=(dc == 0), stop=(dc == DC - 1))
            return xs, mu_ps, msq_ps

        def finish_chunk(ci, xs, mu_ps, msq_ps):
            l0 = ci * NQ
            r_mu = prow.tile([1, NQ], F32, tag="row", name="r_mu")
            nc.vector.tensor_copy(r_mu[:], mu_ps[:])
            r_var = prow.tile([1, NQ], F32, tag="row", name="r_var")
            nc.vector.tensor_mul(r_var[:], r_mu[:], r_mu[:])
            nc.vector.tensor_sub(r_var[:], msq_ps[:], r_var[:])
            r_sd1 = prow.tile([1, NQ], F32, tag="row", name="r_sd1")
            nc.scalar.activation(r_sd1[:], r_var[:], AF.Sqrt, bias=eps1[0:1, 0:1])
            r_rs1 = prow.tile([1, NQ], F32, tag="row", name="r_rs1")
            nc.vector.reciprocal_approx_fast(r_rs1[:], r_sd1[:])
            r_v2 = prow.tile([1, NQ], F32, tag="row", name="r_v2")
            nc.vector.tensor_mul(r_v2[:], r_rs1[:], r_rs1[:])
            nc.vector.tensor_mul(r_v2[:], r_var[:], r_v2[:])
            r_sd2 = prow.tile([1, NQ], F32, tag="row", name="r_sd2")
            nc.scalar.activation(r_sd2[:], r_v2[:], AF.Sqrt, bias=eps1[0:1, 0:1])
            r_S = prow.tile([1, NQ], F32, tag="row", name="r_S")
            nc.vector.reciprocal_approx_fast(r_S[:], r_sd2[:])
            nc.vector.tensor_mul(r_S[:], r_rs1[:], r_S[:])
            mu_ps_b = _bcast(nc, pbc, onesrow, r_mu[:], NQ, "mu_b")
            S_ps_b = _bcast(nc, pbc, onesrow, r_S[:], NQ, "S_b")
            mu_b = ptmp.tile([128, NQ], BF16, tag="mub")
            nc.vector.tensor_copy(mu_b[:], mu_ps_b[:])
            S_b = ptmp.tile([128, NQ], BF16, tag="sb")
            nc.vector.tensor_copy(S_b[:], S_ps_b[:])
            for dc in range(DC):   # all-bf16-SBUF: DVE 2x mode + Pool legal
                eng = nc.vector if dc % 2 == 0 else nc.gpsimd
                tmp = ptmp.tile([128, NQ], BF16, tag="tmpa")
                eng.tensor_sub(tmp[:], xs[dc], mu_b[:])
                eng.tensor_mul(nT[dc][:, l0:l0 + NQ], tmp[:], S_b[:])
            if ci == NCH - 1:
                y_ps_b = _bcast(nc, pbc, onesrow, r_sd2[:], NQ, "y_b")
                y_b = ptmp.tile([128, NQ], BF16, tag="yb")
                nc.vector.tensor_copy(y_b[:], y_ps_b[:])
                for dc in range(DC):
                    eng = nc.vector if dc % 2 == 0 else nc.gpsimd
                    eng.tensor_mul(y1T[dc][:], nT[dc][:, Q0:L], y_b[:])
            for et in range(DC):
                ps = pkps.tile([128, NQ], F32, tag="kps")
                for dc in range(DC):
                    nc.tensor.matmul(
                        ps[:], lhsT=wq[:, dc * 768 + et * 128:dc * 768 + (et + 1) * 128],
                        rhs=nT[dc][:, l0:l0 + NQ],
                        start=(dc == 0), stop=(dc == DC - 1))
                nc.scalar.copy(KT[2 * et][0:64, l0:l0 + NQ], ps[0:64, :])
                nc.vector.tensor_copy(stg[et][64:128, l0:l0 + NQ], ps[64:128, :])

        pend = None   # stats for chunk ci+1 issue before chunk ci's rows/
        for ci in range(NCH):   # nT/K, keeping the PE fed during row math
            cur = stats_mm(ci)
            if pend is not None:
                finish_chunk(ci - 1, *pend)
            pend = cur
        finish_chunk(NCH - 1, *pend)
    for et in range(DC):
        nc.gpsimd.dma_start(KT[2 * et + 1][0:64, :], stg[et][64:128, :])

    with ExitStack() as ps2:
        pkps = ps2.enter_context(tc.tile_pool(name="ab_kps", bufs=2, space="PSUM"))
        pqps = ps2.enter_context(tc.tile_pool(name="ab_qps", bufs=2, space="PSUM"))
        pvps = ps2.enter_context(tc.tile_pool(name="ab_vps", bufs=2, space="PSUM"))
        # K^T: even head lands in its [66, L] tile directly; odd head is
        # staged (engines cannot shift partitions) and DMA'd to partition 0.
        for et in range(DC):
            stg = pstg.tile([128, L], BF16, tag="kstg")
            for ci in range(NCH):
                l0 = ci * NQ
                ps = pkps.tile([128, NQ], F32, tag="kps")
                for dc in range(DC):
                    nc.tensor.matmul(
                        ps[:], lhsT=wq[:, dc * 768 + et * 128:dc * 768 + (et + 1) * 128],
                        rhs=nT[dc][:, l0:l0 + NQ],
                        start=(dc == 0), stop=(dc == DC - 1))
                nc.scalar.copy(KT[2 * et][0:64, l0:l0 + NQ], ps[0:64, :])
                nc.vector.tensor_copy(stg[64:128, l0:l0 + NQ], ps[64:128, :])
            nc.sync.dma_start(KT[2 * et + 1][0:64, :], stg[64:128, :])
        # Q^T with folded bias -> per-head [66, 392] tiles (rows 64/65 gates)
        qstg = pqstg.tile([128, DC * NQ], BF16, tag="qstg")
        for et in range(DC):
            ps = pqps.tile([128, NQ], F32, tag="qps")
            for dc in range(DC):
                nc.tensor.matmul(
                    ps[:], lhsT=wq[:, 4608 + dc * 768 + et * 128:4608 + dc * 768 + (et + 1) * 128],
                    rhs=nT[dc][:, Q0:L], start=(dc == 0), stop=(dc == DC - 1))
            nc.scalar.activation(QT[2 * et][0:64, :], ps[0:64, :], AF.Identity,
                                 bias=bias_sb[0:64, et:et + 1])
            nc.scalar.activation(qstg[64:128, et * NQ:(et + 1) * NQ], ps[64:128, :],
                                 AF.Identity, bias=bias_sb[64:128, et:et + 1])
        for et in range(DC):
            nc.gpsimd.dma_start(QT[2 * et + 1][0:64, :],
                              qstg[64:128, et * NQ:(et + 1) * NQ])

        # V token-major with a ones column per head (softmax denominators)
        for lt in range(NJT):
            lsz = JSZ[lt]
            ps_v = pvps.tile([128, D], F32, tag="psv")
            for dc in range(DC):
                lhsT = nT[dc][:, lt * 128:lt * 128 + lsz]
                rhs = wq[:, 9216 + dc * 768:9216 + (dc + 1) * 768]
                nc.tensor.matmul(ps_v[0:lsz, 0:512], lhsT=lhsT, rhs=rhs[:, 0:512],
                                 start=(dc == 0), stop=(dc == DC - 1),
                                 skip_group_check=True)
                nc.tensor.matmul(ps_v[0:lsz, 512:D], lhsT=lhsT, rhs=rhs[:, 512:D],
                                 start=(dc == 0), stop=(dc == DC - 1),
                                 skip_group_check=True)
            vav = VA[lt][:].rearrange("p (h c) -> p h c", c=65)
            eng = nc.vector if lt % 2 == 0 else nc.gpsimd
            vcv = env["vcolb"][:].rearrange("p (h o) -> p h o", o=1)
            eng.tensor_copy(vav[:, :, 64:65], vcv)
            nc.scalar.copy(vav[0:lsz, :, 0:64],
                           ps_v[0:lsz, :].rearrange("p (h c) -> p h c", c=64))


def _phase_attn(nc, tc, ctx, env):
    """Mask-fused scores (contraction 66), paired exp, PV with ones column,
    batched-reciprocal normalization, then one dense out-projection burst."""
    KT, QT, VA = env["KT"], env["QT"], env["VA"]
    y1T, x2T, bias_sb = env["y1T"], env["x2T"], env["bias_sb"]
    onesrow64 = env["onesrow64"]

    pwD = ctx.enter_context(tc.tile_pool(name="d_w", bufs=1))
    wo = pwD.tile([64, H * D], BF16, tag="wD")
    nc.sync.dma_start(wo[:], env["woutP"][:])
    w1 = env["pw12"].tile([128, DC * DFF], BF16, tag="w1")
    nc.sync.dma_start(w1[:], env["w1P"][:])
    env["w1"] = w1

    pos = ctx.enter_context(tc.tile_pool(name="c_os", bufs=H))
    pds = ctx.enter_context(tc.tile_pool(name="c_ds", bufs=1))
    pont = ctx.enter_context(tc.tile_pool(name="c_ontp", bufs=H))
    ds12 = pds.tile([H, NQ], BF16, tag="ds")
    prcp = None
    oS = [pos.tile([65, NQ], BF16, tag="os", name=f"oS{h}") for h in range(H)]
    ONT = [pont.tile([64, NQ], BF16, tag="ont", name=f"ONT{h}")
           for h in range(H)]

    with ExitStack() as hctx:
        ppt = hctx.enter_context(tc.tile_pool(name="c_pt", bufs=3))
        pss = hctx.enter_context(tc.tile_pool(name="c_ps_s", bufs=3, space="PSUM"))
        pso = hctx.enter_context(tc.tile_pool(name="c_ps_o", bufs=2, space="PSUM"))

        def emit_pv(h, o_ps, jts, pt):
            for k, jt in enumerate(jts):
                nc.tensor.matmul(
                    o_ps[:], lhsT=VA[jt][0:JSZ[jt], h * 65:(h + 1) * 65],
                    rhs=pt[0:JSZ[jt], k * NQ:k * NQ + NQ],
                    start=(jt == 0), stop=(jt == NJT - 1),
                    skip_group_check=True)
            if jts[-1] == NJT - 1:     # head complete: stage it out of PSUM
                nc.vector.tensor_copy(oS[h][:], o_ps[:])
                nc.gpsimd.dma_start(ds12[h:h + 1, :], oS[h][64:65, :])

        pend = None    # 1-deep pipeline ACROSS heads: scores of the next
        for h in range(H):  # pair issue before PV of the current one
            o_ps = pso.tile([65, NQ], F32, tag="pso", name=f"o_ps{h % 2}")
            for jp in range(7):
                jts = [jt for jt in (2 * jp, 2 * jp + 1) if jt < NJT]
                s2 = pss.tile([128, 1024], F32, tag="ps_s")
                for k, jt in enumerate(jts):
                    nc.tensor.matmul(
                        s2[0:JSZ[jt], k * 512:k * 512 + NQ],
                        lhsT=KT[h][0:66, jt * 128:jt * 128 + JSZ[jt]],
                        rhs=QT[h][0:66, :], start=True, stop=True,
                        skip_group_check=True)
                pt = ppt.tile([128, 2 * NQ], BF16, tag="pt")
                if len(jts) == 2:
                    s2v = s2[:].rearrange("p (s c) -> p s c", c=512)[:, :, 0:NQ]
                    ptv = pt[:].rearrange("p (s c) -> p s c", c=NQ)
                    nc.scalar.activation(ptv[:], s2v, AF.Exp)
                else:
                    nc.scalar.activation(pt[:, 0:NQ], s2[:, 0:NQ], AF.Exp)
                if pend is not None:
                    emit_pv(*pend)
                pend = (h, o_ps, jts, pt)
        emit_pv(*pend)

    # batched softmax denominators: one fast reciprocal, flatten to
    # partition 0 by DMA, broadcast back per head via ones-matmuls
    prcp = ctx.enter_context(tc.tile_pool(name="c_rcp", bufs=1))
    dsf = prcp.tile([H, NQ], F32, tag="dsf")
    nc.vector.tensor_copy(dsf[:], ds12[:])
    rcp12 = prcp.tile([H, NQ], F32, tag="rc")
    nc.vector.reciprocal_approx_fast(rcp12[:], dsf[:])
    rcp12b = prcp.tile([H, NQ], BF16, tag="rcb")
    nc.vector.tensor_copy(rcp12b[:], rcp12[:])
    rflat = prcp.tile([1, H * NQ], BF16, tag="rf")
    nc.sync.dma_start(rflat[:], rcp12b[:])
    # FFN2 weights: in queue order after rflat so normalization is not
    # stuck behind 9MB of FFN weights
    w2 = env["pw12"].tile([128, FT * D], BF16, tag="w2")
    nc.sync.dma_start(w2[:], env["w2P"][:])
    env["w2"] = w2

    with ExitStack() as nctx:
        pbc2 = nctx.enter_context(tc.tile_pool(name="c_bc", bufs=2, space="PSUM"))
        pop = nctx.enter_context(tc.tile_pool(name="c_op", bufs=DC, space="PSUM"))
        for h in range(H):      # all normalizations first ...
            bc = pbc2.tile([64, NQ], F32, tag="rb")
            nc.tensor.matmul(bc[:], lhsT=onesrow64[:],
                             rhs=rflat[0:1, h * NQ:(h + 1) * NQ],
                             start=True, stop=True)
            nc.vector.tensor_mul(ONT[h][:], oS[h][0:64, :], bc[:])
        ps_op = [pop.tile([128, NQ], F32, tag="op", name=f"ps_op{i}")
                 for i in range(DC)]
        for h in range(H):      # ... then one uninterrupted matmul burst
            for dt in range(DC):
                nc.tensor.matmul(
                    ps_op[dt][:], lhsT=wo[0:64, h * D + dt * 128:h * D + (dt + 1) * 128],
                    rhs=ONT[h][:], start=(h == 0), stop=(h == H - 1),
                    skip_group_check=True)
        for dt in range(DC):
            nc.vector.scalar_tensor_tensor(
                x2T[dt][:], ps_op[dt][:], bias_sb[:, 6 + dt:7 + dt], y1T[dt][:],
                op0=ALU.add, op1=ALU.add)


def _phase_ffn(nc, tc, ctx, env):
    """LN2 + interleaved FFN1(silu)/FFN2 with residual."""
    onesD32, onesrow, eps1 = env["onesD32"], env["onesrow"], env["eps1"]
    bias_sb = env["bias_sb"]
    x2T, w1, w2 = env["x2T"], env["w1"], env["w2"]

    psq2 = ctx.enter_context(tc.tile_pool(name="e_sq", bufs=3))
    ptmp2 = ctx.enter_context(tc.tile_pool(name="e_tmp", bufs=3))
    prow3 = ctx.enter_context(tc.tile_pool(name="e_rows", bufs=8))
    pn2 = ctx.enter_context(tc.tile_pool(name="e_n2", bufs=DC))
    pffs = ctx.enter_context(tc.tile_pool(name="f_ffs", bufs=2))
    poutT = ctx.enter_context(tc.tile_pool(name="f_outT", bufs=1))

    n2T = [pn2.tile([128, NQ], BF16, tag="n2", name=f"n2T{i}") for i in range(DC)]
    with ExitStack() as lctx:
        pmm2 = lctx.enter_context(tc.tile_pool(name="e_mm", bufs=3, space="PSUM"))
        pbc3 = lctx.enter_context(tc.tile_pool(name="e_bc", bufs=2, space="PSUM"))
        xs = [x2T[dc][:] for dc in range(DC)]
        r_mu2, r_S2, _ = _row_stats(nc, pmm2, prow3, psq2, onesD32, eps1,
                                    xs, NQ, F32R)
        mu2_b = _bcast(nc, pbc3, onesrow, r_mu2[:], NQ, "mu2_b")
        S2_b = _bcast(nc, pbc3, onesrow, r_S2[:], NQ, "S2_b")
        for dc in range(DC):
            tmp = ptmp2.tile([128, NQ], F32, tag="tmp2")
            nc.vector.tensor_sub(tmp[:], x2T[dc][:], mu2_b[:])
            nc.vector.tensor_mul(n2T[dc][:], tmp[:], S2_b[:])

    outT = poutT.tile([128, DC * NQ], F32, tag="outT")
    with ExitStack() as fctx:
        pmmE = fctx.enter_context(tc.tile_pool(name="f_mm", bufs=2, space="PSUM"))
        pacc = fctx.enter_context(tc.tile_pool(name="f_acc", bufs=DC, space="PSUM"))
        ps_acc = [pacc.tile([128, NQ], F32, tag="acc", name=f"ps_acc{i}")
                  for i in range(DC)]
        for ft in range(FT):
            ps1 = pmmE.tile([128, NQ], F32, tag="mm", name="ps1")
            for dc in range(DC):
                nc.tensor.matmul(
                    ps1[:], lhsT=w1[:, dc * DFF + ft * 128:dc * DFF + (ft + 1) * 128],
                    rhs=n2T[dc][:], start=(dc == 0), stop=(dc == DC - 1))
            # silu(u) = u * sigmoid(u) with u = ps1 + cb1
            sig = pffs.tile([128, NQ], BF16, tag="sig")
            nc.scalar.activation(sig[:], ps1[:], AF.Sigmoid,
                                 bias=bias_sb[:, 18 + ft:19 + ft])
            ffs = pffs.tile([128, NQ], BF16, tag="ffs")
            nc.vector.scalar_tensor_tensor(ffs[:], ps1[:], bias_sb[:, 18 + ft:19 + ft],
                                           sig[:], op0=ALU.add, op1=ALU.mult)
            for dt in range(DC):
                nc.tensor.matmul(
                    ps_acc[dt][:], lhsT=w2[:, ft * D + dt * 128:ft * D + (dt + 1) * 128],
                    rhs=ffs[:], start=(ft == 0), stop=(ft == FT - 1),
                    skip_group_check=True)
        for dt in range(DC):
            nc.vector.scalar_tensor_tensor(
                outT[:, dt * NQ:(dt + 1) * NQ], ps_acc[dt][:],
                bias_sb[:, 12 + dt:13 + dt], x2T[dt][:],
                op0=ALU.add, op1=ALU.add)
    nc.sync.dma_start(env["out"][:], outT[:])


def build_program():
    nc = bacc.Bacc("TRN2")
    env = {}
    env["xp"] = nc.declare_dram_parameter("xp", [128, NCH * DC * NQ], BF16, isOutput=False)
    env["wqkvP"] = nc.declare_dram_parameter("wqkvP", [128, 3 * 4608], BF16, isOutput=False)
    env["mskr"] = nc.declare_dram_parameter("mskr", [2, L], BF16, isOutput=False)
    env["qg"] = nc.declare_dram_parameter("qg", [2, NQ], BF16, isOutput=False)
    env["woutP"] = nc.declare_dram_parameter("woutP", [64, H * D], BF16, isOutput=False)
    env["w1P"] = nc.declare_dram_parameter("w1P", [128, DC * DFF], BF16, isOutput=False)
    env["w2P"] = nc.declare_dram_parameter("w2P", [128, FT * D], BF16, isOutput=False)
    biasP = nc.declare_dram_parameter("biasP", [128, 42], F32, isOutput=False)
    env["out"] = nc.declare_dram_parameter("out", [128, DC * NQ], F32, isOutput=True)

    with tile.TileContext(nc) as tc, ExitStack() as top:
        pc = top.enter_context(tc.tile_pool(name="const", bufs=1))
        px2 = top.enter_context(tc.tile_pool(name="x2p", bufs=DC))

        onesf = pc.tile([128, 1], F32, tag="onesf")
        nc.vector.memset(onesf[:], 1.0 / D)
        ones = pc.tile([128, 1], BF16, tag="ones")
        nc.vector.tensor_copy(ones[:], onesf[:])
        onesD32 = pc.tile([128, 1], F32R, tag="ones32")
        nc.vector.tensor_copy(onesD32[:], onesf[:])
        onesrow = pc.tile([1, 128], F32, tag="onesrow")
        nc.vector.memset(onesrow[:], 1.0)
        onesrow64 = pc.tile([1, 64], BF16, tag="onesrow64")
        nc.vector.tensor_copy(onesrow64[:], onesrow[0:1, 0:64])
        eps1 = pc.tile([1, 1], F32, tag="eps1")
        nc.vector.memset(eps1[:], EPS)
        vcolf = pc.tile([128, H], F32, tag="vcolf")
        nc.vector.memset(vcolf[:], 1.0)
        vcolb = pc.tile([128, H], BF16, tag="vcolb")
        nc.vector.tensor_copy(vcolb[:], vcolf[:])
        env["vcolb"] = vcolb
        bias_sb = pc.tile([128, 42], F32, tag="bias")
        nc.sync.dma_start(bias_sb[:], biasP[:])
        env.update(ones=ones, onesD32=onesD32, onesrow=onesrow,
                   onesrow64=onesrow64, eps1=eps1, bias_sb=bias_sb)

        env["x2T"] = [px2.tile([128, NQ], F32R, tag="x2", name=f"x2T{i}")
                      for i in range(DC)]

        with ExitStack() as mid:
            pkt = mid.enter_context(tc.tile_pool(name="ktp", bufs=H))
            pqt = mid.enter_context(tc.tile_pool(name="qtp", bufs=H))
            pva = mid.enter_context(tc.tile_pool(name="vap", bufs=NJT))
            py1 = mid.enter_context(tc.tile_pool(name="y1p", bufs=DC))
            env["KT"] = [pkt.tile([66, L], BF16, tag="kt", name=f"KT{i}")
                         for i in range(H)]
            env["QT"] = [pqt.tile([66, NQ], BF16, tag="qt", name=f"QT{i}")
                         for i in range(H)]
            env["VA"] = [pva.tile([128, H * 65], BF16, tag="va", name=f"VA{i}")
                         for i in range(NJT)]
            env["y1T"] = [py1.tile([128, NQ], BF16, tag="y1", name=f"y1T{i}")
                          for i in range(DC)]

            with ExitStack() as ctx:
                _phase_ab(nc, tc, ctx, env)

            # FFN weight pools open late: during phase AB they would starve
            # SBUF; w2's opens mid-attention. Both outlive the attention scope.
            env["pw12"] = mid.enter_context(tc.tile_pool(name="w12p", bufs=1))
            env["mid"] = mid

            with ExitStack() as ctx:
                _phase_attn(nc, tc, ctx, env)

            with ExitStack() as ctx:
                _phase_ffn(nc, tc, ctx, env)

    nc.finalize()
    return nc


_NC = None


def _get_nc():
    global _NC
    if _NC is None:
        _NC = build_program()
    return _NC


def _host_prepare(inputs):
    """Fold constants and lay out per-core input maps (pure layout work)."""
    f32 = np.float32
    x = np.asarray(inputs["x"], f32)
    memory = np.asarray(inputs["memory"], f32)
    w_qkv = np.asarray(inputs["w_qkv"], f32)
    w_out = np.asarray(inputs["w_out"], f32)
    b_out = np.asarray(inputs["b_out"], f32)
    g_att = np.asarray(inputs["ln_att_g"], f32)
    b_att = np.asarray(inputs["ln_att_b"], f32)
    g2 = np.asarray(inputs["ln2_g"], f32)
    bb2 = np.asarray(inputs["ln2_b"], f32)
    w1 = np.asarray(inputs["w1"], f32)
    b1 = np.asarray(inputs["b1"], f32)
    w2 = np.asarray(inputs["w2"], f32)
    b2v = np.asarray(inputs["b2"], f32)

    qscale = f32(64 ** -0.5)
    w_qkv_eff = w_qkv * g_att[None, :]
    w_qkv_eff[:D] *= qscale
    cb_qkv = w_qkv @ b_att
    cb_q = (cb_qkv[:D] * qscale).astype(f32)
    cb_v = cb_qkv[2 * D:].astype(f32)
    b_out_eff = (b_out + w_out @ cb_v).astype(f32)
    w1_eff = w1 * g2[None, :]
    cb1_eff = (w1 @ bb2 + b1).astype(f32)

    def cols(v):
        return np.ascontiguousarray(v.reshape(-1, 128).T)

    biasP = np.zeros((128, 42), f32)
    biasP[:, 0:6] = cols(cb_q)
    biasP[:, 6:12] = cols(b_out_eff)
    biasP[:, 12:18] = cols(b2v)
    biasP[:, 18:42] = cols(cb1_eff)

    def packP(wT, ncol):
        # [D_in, ncol] -> [128, (D_in/128)*ncol] partition-packed bf16
        return np.ascontiguousarray(
            wT.reshape(-1, 128, ncol).transpose(1, 0, 2).reshape(128, -1)
        ).astype(NPBF16)

    wq_T = np.ascontiguousarray(w_qkv_eff.T)       # [D, 3D]
    wqkvP = np.concatenate(
        [packP(np.ascontiguousarray(wq_T[:, D:2 * D]), D),      # K
         packP(np.ascontiguousarray(wq_T[:, 0:D]), D),          # Q
         packP(np.ascontiguousarray(wq_T[:, 2 * D:3 * D]), D)], # V
        axis=1)
    woutP = np.ascontiguousarray(
        w_out.T.reshape(H, 64, D).transpose(1, 0, 2).reshape(64, H * D)
    ).astype(NPBF16)

    shared = {
        "wqkvP": wqkvP,
        "woutP": woutP,
        "w1P": packP(np.ascontiguousarray(w1_eff.T), DFF),
        "w2P": packP(np.ascontiguousarray(w2.T), D),
        "biasP": biasP,
    }

    perm0 = np.concatenate([np.arange(0, T), np.arange(Q0, L), np.arange(T, Q0)])
    in_maps = []
    for c in range(NCORES):
        b, hf = divmod(c, 2)
        x_aug = np.concatenate([memory[b, :T], x[b]], axis=0)      # [L, D]
        old = perm0 if hf == 0 else np.arange(L)
        xa = x_aug[old]
        # [p, ci*6*392 + dc*392 + q] = xa[ci*392+q, dc*128+p]
        xp = np.ascontiguousarray(
            xa.T.reshape(DC, 128, NCH, NQ).transpose(1, 2, 0, 3).reshape(128, -1)
        ).astype(NPBF16)
        LcA = (5 + 2 * hf) * NPATCH
        LcB = (6 + 2 * hf) * NPATCH
        mb = np.where(old < LcB, 0.0, MASKB).astype(f32)
        ma = np.where(old < LcA, 0.0, MASKB).astype(f32)
        mskr = np.stack([mb, ma - mb]).astype(NPBF16)
        qg = np.stack([np.ones(NQ, f32),
                       (np.arange(NQ) < NPATCH).astype(f32)]).astype(NPBF16)
        in_maps.append({"xp": xp, "mskr": mskr, "qg": qg, **shared})
    return in_maps


def _assemble(results):
    out = np.zeros((B, T, D), np.float32)
    for c in range(NCORES):
        b, hf = divmod(c, 2)
        fm = results[c]["out"].reshape(128, DC, NQ).transpose(1, 0, 2).reshape(D, NQ)
        out[b, hf * NQ:(hf + 1) * NQ, :] = fm.T
    return out


def kernel(**inputs):
    nc = _get_nc()
    in_maps = _host_prepare(inputs)
    res = run_bass_kernel_spmd(nc, in_maps, list(range(NCORES)))
    return _assemble(res.results)


def _ensure_ntff_hook():
    """Provide antenv.axon_hooks (absent in this image) so trace=True can
    drive NTFF capture through libaxon_pjrt.so, mirroring trn_boot.py."""
    import contextlib
    import ctypes
    import types

    try:
        from antenv.axon_hooks import get_axon_ntff_profile_hook  # noqa: F401
        return
    except ImportError:
        pass
    import antenv

    so_path = "/opt/axon/libaxon_pjrt.so"
    lib = ctypes.CDLL(so_path)
    if not hasattr(lib, "axon_start_nrt_profile"):
        raise RuntimeError("libaxon_pjrt.so lacks NTFF profile symbols")
    lib.axon_start_nrt_profile.argtypes = [ctypes.POINTER(ctypes.c_int64),
                                           ctypes.c_size_t]
    lib.axon_start_nrt_profile.restype = ctypes.c_int64
    lib.axon_stop_nrt_profile.argtypes = [ctypes.c_char_p]
    lib.axon_stop_nrt_profile.restype = ctypes.c_int64

    @contextlib.contextmanager
    def _hook(output_dir, device_ids):
        import jax
        jax.devices()
        if device_ids:
            ids = (ctypes.c_int64 * len(device_ids))(*device_ids)
            rc = lib.axon_start_nrt_profile(ids, len(device_ids))
        else:
            rc = lib.axon_start_nrt_profile(None, 0)
        if rc != 0:
            raise RuntimeError(f"axon_start_nrt_profile rc={rc}")
        try:
            yield
        finally:
            n = lib.axon_stop_nrt_profile(str(output_dir).encode())
            print(f"ntff profile: {n} file(s) written to {output_dir}",
                  file=sys.stderr)

    box = {"h": _hook}
    mod = types.ModuleType("antenv.axon_hooks")
    mod.set_axon_ntff_profile_hook = lambda h: box.__setitem__("h", h)
    mod.get_axon_ntff_profile_hook = lambda: box["h"]
    sys.modules["antenv.axon_hooks"] = mod
    antenv.axon_hooks = mod


def kernel_traced(**inputs):
    """Like kernel() but with NTFF profiling; returns (out, exec_time_ns)."""
    import tempfile

    from concourse import bass_utils as _bu
    _ensure_ntff_hook()
    _bu.upload_artifacts = lambda tmpdir: f"local:{tmpdir}"  # no bucket creds here
    nc = _get_nc()
    in_maps = _host_prepare(inputs)
    tmpdir = tempfile.mkdtemp(prefix="ntff_")
    res = run_bass_kernel_spmd(nc, in_maps, list(range(NCORES)), trace=True,
                               tmpdir=tmpdir)
    return _assemble(res.results), res.exec_time_ns


# revision 27
# speedup vs baseline: 1.0872x; 1.0244x over previous
"""Trainium2 Bass kernel: LookupTransformerBlock (block-causal sparse attention).

Reference semantics (B=4, T=784, D=768, H=12, Dh=64, d_ff=3072):
  x_aug = LN1(concat(memory[:, :T], x))              # [B, 2T, D]
  h     = LN_att(x_aug)
  qkv   = h @ w_qkv.T ; block-causal attention over frames of 196
  x2    = x_aug + attn_out
  out   = (x2 + FFN(LN2(x2)))[:, T:, :]

Sharding: 8 cores = (batch b in 0..3) x (query-half hf in 0..1); each core
computes its 392 output rows with K/V over all 1568 positions (data-parallel,
no collectives).  One SPMD program; per-core differences live in input data.

Layout decisions (driven by the DMA/engine-overhead analysis of the previous
version's trace — 500+ small DMAs and per-tile mask exps were the bottleneck):
  - every DRAM tensor is packed host-side so a handful of dma_starts move
    everything with multi-KB per-partition contiguous rows, weights and
    activations in bf16;
  - the host permutes key columns per core so the query slice is always
    columns [1176:1568) -> LN1 chunk-3 stats double as the query stats and
    q-tiles are plain slices;
  - the attention mask is folded into the score matmul: per-head K tiles
    carry two extra contraction rows (frame-B mask bias, frame-A correction)
    and per-head Q tiles the matching gate rows (1, 1_{query in frame A}).
    Exp is then mask-free, so score j-tiles are exp'd in pairs out of PSUM;
  - softmax denominators come from a ones column in V, leave PSUM through
    per-head row DMAs, hit one batched reciprocal_approx_fast, and return
    as PE ones-matmul broadcasts (no DRAM bounces anywhere);
  - the final output stays feature-major on device; the host transposes.
"""

import os
import sys
from contextlib import ExitStack

import numpy as np
import ml_dtypes

for _p in ("/opt/trn_rl_repo", os.path.expanduser("~/.axon_site/_ro/trn_rl_repo")):
    if os.path.isdir(_p) and _p not in sys.path:
        sys.path.append(_p)

import concourse.bass as bass
import concourse.bacc as bacc
import concourse.mybir as mybir
import concourse.tile as tile
from concourse.bass_utils import run_bass_kernel_spmd

F32 = mybir.dt.float32
F32R = mybir.dt.float32r
BF16 = mybir.dt.bfloat16
AF = mybir.ActivationFunctionType
ALU = mybir.AluOpType
NPBF16 = ml_dtypes.bfloat16

B = 4
T = 784
D = 768
L = 2 * T            # 1568
NQ = 392             # query rows per core (= LN chunk width)
Q0 = L - NQ          # queries always live at columns [1176:1568)
H = 12
DFF = 3072
NPATCH = 196
DC = D // 128        # 6
FT = DFF // 128      # 24
NJT = 13             # j-tiles over L (12 x 128 + 32)
JSZ = [128] * 12 + [32]
NCH = 4              # LN chunks, 4 x 392
EPS = 1e-5
NCORES = 8
MASKB = -40.0        # additive mask bias (exp(s-40) ~ 1e-16)


def _row_stats(nc, pmm, prow, psq, ones, eps1, xs, n, sqdt):
    """Column mean / fused-LN scale for feature-major tiles xs (6 x [128,n]).

    Returns rows (r_mu, r_S, r_sd2) with S = rs1*rs2 the fused LN1+LN_att
    scale and sd2 = 1/rs2 (the y1-path scale)."""
    mu_ps = pmm.tile([1, n], F32, tag="mm", name="mu_ps")
    msq_ps = pmm.tile([1, n], F32, tag="mm", name="msq_ps")
    for dc in range(DC):
        nc.tensor.matmul(mu_ps[:], lhsT=ones[:], rhs=xs[dc],
                         start=(dc == 0), stop=(dc == DC - 1))
    for dc in range(DC):
        sq = psq.tile([128, n], sqdt, tag="sq")
        eng = nc.vector if dc % 3 != 2 else nc.gpsimd
        eng.tensor_mul(sq[:], xs[dc], xs[dc])
        nc.tensor.matmul(msq_ps[:], lhsT=ones[:], rhs=sq[:],
                         start=(dc == 0), stop=(dc == DC - 1))
    r_mu = prow.tile([1, n], F32, tag="row", name="r_mu")
    nc.vector.tensor_copy(r_mu[:], mu_ps[:])
    r_var = prow.tile([1, n], F32, tag="row", name="r_var")
    nc.vector.tensor_mul(r_var[:], r_mu[:], r_mu[:])
    nc.vector.tensor_sub(r_var[:], msq_ps[:], r_var[:])
    r_sd1 = prow.tile([1, n], F32, tag="row", name="r_sd1")
    nc.scalar.activation(r_sd1[:], r_var[:], AF.Sqrt, bias=eps1[0:1, 0:1])
    r_rs1 = prow.tile([1, n], F32, tag="row", name="r_rs1")
    nc.vector.reciprocal_approx_fast(r_rs1[:], r_sd1[:])
    r_v2 = prow.tile([1, n], F32, tag="row", name="r_v2")
    nc.vector.tensor_mul(r_v2[:], r_rs1[:], r_rs1[:])
    nc.vector.tensor_mul(r_v2[:], r_var[:], r_v2[:])      # var2 = var*rs1^2
    r_sd2 = prow.tile([1, n], F32, tag="row", name="r_sd2")
    nc.scalar.activation(r_sd2[:], r_v2[:], AF.Sqrt, bias=eps1[0:1, 0:1])
    r_S = prow.tile([1, n], F32, tag="row", name="r_S")
    nc.vector.reciprocal_approx_fast(r_S[:], r_sd2[:])
    nc.vector.tensor_mul(r_S[:], r_rs1[:], r_S[:])        # S = rs1*rs2
    return r_mu, r_S, r_sd2


def _bcast(nc, pbc, onesrow, row, n, name="bc"):
    """[1, n] SBUF fp32 row -> [128, n] PSUM via ones-matmul broadcast."""
    bc = pbc.tile([128, n], F32, tag="bc", name=name)
    nc.tensor.matmul(bc[:], lhsT=onesrow[:], rhs=row, start=True, stop=True)
    return bc


def _phase_ab(nc, tc, ctx, env):
    """LN1+LN_att fused normalization, then K/Q/V GEMMs into per-head tiles."""
    xp, wqkvP, mskr = env["xp"], env["wqkvP"], env["mskr"]
    ones, onesrow, eps1 = env["ones"], env["onesrow"], env["eps1"]
    KT, QT, VA, y1T = env["KT"], env["QT"], env["VA"], env["y1T"]
    bias_sb = env["bias_sb"]

    pxp = ctx.enter_context(tc.tile_pool(name="ab_x", bufs=3))
    pw = ctx.enter_context(tc.tile_pool(name="ab_w", bufs=1))
    pnt = ctx.enter_context(tc.tile_pool(name="ab_nt", bufs=DC))
    psq = ctx.enter_context(tc.tile_pool(name="ab_sq", bufs=2))
    ptmp = ctx.enter_context(tc.tile_pool(name="ab_tmp", bufs=2))
    prow = ctx.enter_context(tc.tile_pool(name="ab_rows", bufs=7))
    pstg = ctx.enter_context(tc.tile_pool(name="ab_stg", bufs=DC))
    pqstg = ctx.enter_context(tc.tile_pool(name="ab_qstg", bufs=1))

    # few large DMAs, interleaved so chunk-0 stats and K weights land first
    # (xc chunks 2/3 reuse chunk-0/1 buffers, so their DMAs go last in the
    # queue: they block on chunk-0/1 reads completing)
    wq = pw.tile([128, 3 * 4608], BF16, tag="wqkv")
    xc = [pxp.tile([128, DC * NQ], BF16, tag="xp", name=f"xp{ci}")
          for ci in range(NCH)]
    nc.sync.dma_start(xc[0][:], xp[:, 0:DC * NQ])
    nc.sync.dma_start(wq[:, 0:4608], wqkvP[:, 0:4608])              # K block
    nc.sync.dma_start(xc[1][:], xp[:, DC * NQ:2 * DC * NQ])
    nc.sync.dma_start(xc[2][:], xp[:, 2 * DC * NQ:3 * DC * NQ])
    nc.sync.dma_start(wq[:, 4608:9216], wqkvP[:, 4608:9216])        # Q block
    nc.sync.dma_start(wq[:, 9216:13824], wqkvP[:, 9216:13824])      # V block
    nc.sync.dma_start(xc[3][:], xp[:, 3 * DC * NQ:4 * DC * NQ])
    # mask/gate rows ride the idle GpSimd DGE queue so they never delay
    # the bulk input stream on the sync queue
    for h in range(H):
        nc.sync.dma_start(KT[h][64:66, :], mskr[:])
        nc.sync.dma_start(QT[h][64:66, :], env["qg"][:])

    nT = [pnt.tile([128, L], BF16, tag="nt", name=f"nT{i}") for i in range(DC)]
    stg = [pstg.tile([128, L], BF16, tag="kstg", name=f"kstg{i}")
           for i in range(DC)]

    with ExitStack() as ps1:
        pmm = ps1.enter_context(tc.tile_pool(name="ab_mm", bufs=4, space="PSUM"))
        pbc = ps1.enter_context(tc.tile_pool(name="ab_bc", bufs=2, space="PSUM"))
        pkps = ps1.enter_context(tc.tile_pool(name="ab_kps", bufs=2, space="PSUM"))

        def stats_mm(ci):
            xs = [xc[ci][:, dc * NQ:(dc + 1) * NQ] for dc in range(DC)]
            mu_ps = pmm.tile([1, NQ], F32, tag="mm", name="mu_ps")
            msq_ps = pmm.tile([1, NQ], F32, tag="mm", name="msq_ps")
            for dc in range(DC):
                nc.tensor.matmul(mu_ps[:], lhsT=ones[:], rhs=xs[dc],
                                 start=(dc == 0), stop=(dc == DC - 1))
            for dc in range(DC):
                sq = psq.tile([128, NQ], BF16, tag="sq")
                eng = nc.vector if dc % 3 != 2 else nc.gpsimd
                eng.tensor_mul(sq[:], xs[dc], xs[dc])
                nc.tensor.matmul(msq_ps[:], lhsT=ones[:], rhs=sq[:],
                                 start=(dc == 0), stop=(dc == DC - 1))
            return xs, mu_ps, msq_ps

        def finish_chunk(ci, xs, mu_ps, msq_ps):
            l0 = ci * NQ
            r_mu = prow.tile([1, NQ], F32, tag="row", name="r_mu")
            nc.vector.tensor_copy(r_mu[:], mu_ps[:])
            r_var = prow.tile([1, NQ], F32, tag="row", name="r_var")
            nc.vector.tensor_mul(r_var[:], r_mu[:], r_mu[:])
            nc.vector.tensor_sub(r_var[:], msq_ps[:], r_var[:])
            r_sd1 = prow.tile([1, NQ], F32, tag="row", name="r_sd1")
            nc.scalar.activation(r_sd1[:], r_var[:], AF.Sqrt, bias=eps1[0:1, 0:1])
            r_rs1 = prow.tile([1, NQ], F32, tag="row", name="r_rs1")
            nc.vector.reciprocal_approx_fast(r_rs1[:], r_sd1[:])
            r_v2 = prow.tile([1, NQ], F32, tag="row", name="r_v2")
            nc.vector.tensor_mul(r_v2[:], r_rs1[:], r_rs1[:])
            nc.vector.tensor_mul(r_v2[:], r_var[:], r_v2[:])
            r_sd2 = prow.tile([1, NQ], F32, tag="row", name="r_sd2")
            nc.scalar.activation(r_sd2[:], r_v2[:], AF.Sqrt, bias=eps1[0:1, 0:1])
            r_S = prow.tile([1, NQ], F32, tag="row", name="r_S")
            nc.vector.reciprocal_approx_fast(r_S[:], r_sd2[:])
            nc.vector.tensor_mul(r_S[:], r_rs1[:], r_S[:])
            mu_ps_b = _bcast(nc, pbc, onesrow, r_mu[:], NQ, "mu_b")
            S_ps_b = _bcast(nc, pbc, onesrow, r_S[:], NQ, "S_b")
            mu_b = ptmp.tile([128, NQ], BF16, tag="mub")
            nc.vector.tensor_copy(mu_b[:], mu_ps_b[:])
            S_b = ptmp.tile([128, NQ], BF16, tag="sb")
            nc.vector.tensor_copy(S_b[:], S_ps_b[:])
            for dc in range(DC):   # all-bf16-SBUF: DVE 2x mode + Pool legal
                eng = nc.vector if dc % 2 == 0 else nc.gpsimd
                tmp = ptmp.tile([128, NQ], BF16, tag="tmpa")
                eng.tensor_sub(tmp[:], xs[dc], mu_b[:])
                eng.tensor_mul(nT[dc][:, l0:l0 + NQ], tmp[:], S_b[:])
            if ci == NCH - 1:
                y_ps_b = _bcast(nc, pbc, onesrow, r_sd2[:], NQ, "y_b")
                y_b = ptmp.tile([128, NQ], BF16, tag="yb")
                nc.vector.tensor_copy(y_b[:], y_ps_b[:])
                for dc in range(DC):
                    eng = nc.vector if dc % 2 == 0 else nc.gpsimd
                    eng.tensor_mul(y1T[dc][:], nT[dc][:, Q0:L], y_b[:])
            for et in range(DC):
                ps = pkps.tile([128, NQ], F32, tag="kps")
                for dc in range(DC):
                    nc.tensor.matmul(
                        ps[:], lhsT=wq[:, dc * 768 + et * 128:dc * 768 + (et + 1) * 128],
                        rhs=nT[dc][:, l0:l0 + NQ],
                        start=(dc == 0), stop=(dc == DC - 1))
                nc.scalar.copy(KT[2 * et][0:64, l0:l0 + NQ], ps[0:64, :])
                nc.vector.tensor_copy(stg[et][64:128, l0:l0 + NQ], ps[64:128, :])

        pend = None   # stats for chunk ci+1 issue before chunk ci's rows/
        for ci in range(NCH):   # nT/K, keeping the PE fed during row math
            cur = stats_mm(ci)
            if pend is not None:
                finish_chunk(ci - 1, *pend)
            pend = cur
        finish_chunk(NCH - 1, *pend)
    for et in range(DC):
        nc.sync.dma_start(KT[2 * et + 1][0:64, :], stg[et][64:128, :])

    with ExitStack() as ps2:
        pkps = ps2.enter_context(tc.tile_pool(name="ab_kps", bufs=2, space="PSUM"))
        pqps = ps2.enter_context(tc.tile_pool(name="ab_qps", bufs=2, space="PSUM"))
        pvps = ps2.enter_context(tc.tile_pool(name="ab_vps", bufs=2, space="PSUM"))
        # K^T: even head lands in its [66, L] tile directly; odd head is
        # staged (engines cannot shift partitions) and DMA'd to partition 0.
        for et in range(DC):
            stg = pstg.tile([128, L], BF16, tag="kstg")
            for ci in range(NCH):
                l0 = ci * NQ
                ps = pkps.tile([128, NQ], F32, tag="kps")
                for dc in range(DC):
                    nc.tensor.matmul(
                        ps[:], lhsT=wq[:, dc * 768 + et * 128:dc * 768 + (et + 1) * 128],
                        rhs=nT[dc][:, l0:l0 + NQ],
                        start=(dc == 0), stop=(dc == DC - 1))
                nc.scalar.copy(KT[2 * et][0:64, l0:l0 + NQ], ps[0:64, :])
                nc.vector.tensor_copy(stg[64:128, l0:l0 + NQ], ps[64:128, :])
            nc.sync.dma_start(KT[2 * et + 1][0:64, :], stg[64:128, :])
        # Q^T with folded bias -> per-head [66, 392] tiles (rows 64/65 gates)
        qstg = pqstg.tile([128, DC * NQ], BF16, tag="qstg")
        for et in range(DC):
            ps = pqps.tile([128, NQ], F32, tag="qps")
            for dc in range(DC):
                nc.tensor.matmul(
                    ps[:], lhsT=wq[:, 4608 + dc * 768 + et * 128:4608 + dc * 768 + (et + 1) * 128],
                    rhs=nT[dc][:, Q0:L], start=(dc == 0), stop=(dc == DC - 1))
            nc.scalar.activation(QT[2 * et][0:64, :], ps[0:64, :], AF.Identity,
                                 bias=bias_sb[0:64, et:et + 1])
            nc.scalar.activation(qstg[64:128, et * NQ:(et + 1) * NQ], ps[64:128, :],
                                 AF.Identity, bias=bias_sb[64:128, et:et + 1])
        for et in range(DC):
            nc.sync.dma_start(QT[2 * et + 1][0:64, :],
                              qstg[64:128, et * NQ:(et + 1) * NQ])

        # V token-major with a ones column per head (softmax denominators)
        for lt in range(NJT):
            lsz = JSZ[lt]
            ps_v = pvps.tile([128, D], F32, tag="psv")
            for dc in range(DC):
                lhsT = nT[dc][:, lt * 128:lt * 128 + lsz]
                rhs = wq[:, 9216 + dc * 768:9216 + (dc + 1) * 768]
                nc.tensor.matmul(ps_v[0:lsz, 0:512], lhsT=lhsT, rhs=rhs[:, 0:512],
                                 start=(dc == 0), stop=(dc == DC - 1),
                                 skip_group_check=True)
                nc.tensor.matmul(ps_v[0:lsz, 512:D], lhsT=lhsT, rhs=rhs[:, 512:D],
                                 start=(dc == 0), stop=(dc == DC - 1),
                                 skip_group_check=True)
            vav = VA[lt][:].rearrange("p (h c) -> p h c", c=65)
            eng = nc.vector if lt % 2 == 0 else nc.gpsimd
            vcv = env["vcolb"][:].rearrange("p (h o) -> p h o", o=1)
            eng.tensor_copy(vav[:, :, 64:65], vcv)
            nc.scalar.copy(vav[0:lsz, :, 0:64],
                           ps_v[0:lsz, :].rearrange("p (h c) -> p h c", c=64))


def _phase_attn(nc, tc, ctx, env):
    """Mask-fused scores (contraction 66), paired exp, PV with ones column,
    batched-reciprocal normalization, then one dense out-projection burst."""
    KT, QT, VA = env["KT"], env["QT"], env["VA"]
    y1T, x2T, bias_sb = env["y1T"], env["x2T"], env["bias_sb"]
    onesrow64 = env["onesrow64"]

    pwD = ctx.enter_context(tc.tile_pool(name="d_w", bufs=1))
    wo = pwD.tile([64, H * D], BF16, tag="wD")
    nc.sync.dma_start(wo[:], env["woutP"][:])
    w1 = env["pw12"].tile([128, DC * DFF], BF16, tag="w1")
    nc.sync.dma_start(w1[:], env["w1P"][:])
    env["w1"] = w1

    pos = ctx.enter_context(tc.tile_pool(name="c_os", bufs=H))
    pds = ctx.enter_context(tc.tile_pool(name="c_ds", bufs=1))
    pont = ctx.enter_context(tc.tile_pool(name="c_ontp", bufs=H))
    ds12 = pds.tile([H, NQ], BF16, tag="ds")
    prcp = None
    oS = [pos.tile([65, NQ], BF16, tag="os", name=f"oS{h}") for h in range(H)]
    ONT = [pont.tile([64, NQ], BF16, tag="ont", name=f"ONT{h}")
           for h in range(H)]

    with ExitStack() as hctx:
        ppt = hctx.enter_context(tc.tile_pool(name="c_pt", bufs=3))
        pss = hctx.enter_context(tc.tile_pool(name="c_ps_s", bufs=3, space="PSUM"))
        pso = hctx.enter_context(tc.tile_pool(name="c_ps_o", bufs=2, space="PSUM"))

        def emit_pv(h, o_ps, jts, pt):
            for k, jt in enumerate(jts):
                nc.tensor.matmul(
                    o_ps[:], lhsT=VA[jt][0:JSZ[jt], h * 65:(h + 1) * 65],
                    rhs=pt[0:JSZ[jt], k * NQ:k * NQ + NQ],
                    start=(jt == 0), stop=(jt == NJT - 1),
                    skip_group_check=True)
            if jts[-1] == NJT - 1:     # head complete: stage it out of PSUM
                nc.vector.tensor_copy(oS[h][:], o_ps[:])
                nc.sync.dma_start(ds12[h:h + 1, :], oS[h][64:65, :])

        pend = None    # 1-deep pipeline ACROSS heads: scores of the next
        for h in range(H):  # pair issue before PV of the current one
            o_ps = pso.tile([65, NQ], F32, tag="pso", name=f"o_ps{h % 2}")
            for jp in range(7):
                jts = [jt for jt in (2 * jp, 2 * jp + 1) if jt < NJT]
                s2 = pss.tile([128, 1024], F32, tag="ps_s")
                for k, jt in enumerate(jts):
                    nc.tensor.matmul(
                        s2[0:JSZ[jt], k * 512:k * 512 + NQ],
                        lhsT=KT[h][0:66, jt * 128:jt * 128 + JSZ[jt]],
                        rhs=QT[h][0:66, :], start=True, stop=True,
                        skip_group_check=True)
                pt = ppt.tile([128, 2 * NQ], BF16, tag="pt")
                if len(jts) == 2:
                    s2v = s2[:].rearrange("p (s c) -> p s c", c=512)[:, :, 0:NQ]
                    ptv = pt[:].rearrange("p (s c) -> p s c", c=NQ)
                    nc.scalar.activation(ptv[:], s2v, AF.Exp)
                else:
                    nc.scalar.activation(pt[:, 0:NQ], s2[:, 0:NQ], AF.Exp)
                if pend is not None:
                    emit_pv(*pend)
                pend = (h, o_ps, jts, pt)
        emit_pv(*pend)

    # batched softmax denominators: one fast reciprocal, flatten to
    # partition 0 by DMA, broadcast back per head via ones-matmuls
    prcp = ctx.enter_context(tc.tile_pool(name="c_rcp", bufs=1))
    dsf = prcp.tile([H, NQ], F32, tag="dsf")
    nc.vector.tensor_copy(dsf[:], ds12[:])
    rcp12 = prcp.tile([H, NQ], F32, tag="rc")
    nc.vector.reciprocal_approx_fast(rcp12[:], dsf[:])
    rcp12b = prcp.tile([H, NQ], BF16, tag="rcb")
    nc.vector.tensor_copy(rcp12b[:], rcp12[:])
    rflat = prcp.tile([1, H * NQ], BF16, tag="rf")
    nc.sync.dma_start(rflat[:], rcp12b[:])
    # FFN2 weights: in queue order after rflat so normalization is not
    # stuck behind 9MB of FFN weights
    w2 = env["pw12"].tile([128, FT * D], BF16, tag="w2")
    nc.sync.dma_start(w2[:], env["w2P"][:])
    env["w2"] = w2

    with ExitStack() as nctx:
        pbc2 = nctx.enter_context(tc.tile_pool(name="c_bc", bufs=2, space="PSUM"))
        pop = nctx.enter_context(tc.tile_pool(name="c_op", bufs=DC, space="PSUM"))
        for h in range(H):      # all normalizations first ...
            bc = pbc2.tile([64, NQ], F32, tag="rb")
            nc.tensor.matmul(bc[:], lhsT=onesrow64[:],
                             rhs=rflat[0:1, h * NQ:(h + 1) * NQ],
                             start=True, stop=True)
            nc.vector.tensor_mul(ONT[h][:], oS[h][0:64, :], bc[:])
        ps_op = [pop.tile([128, NQ], F32, tag="op", name=f"ps_op{i}")
                 for i in range(DC)]
        for h in range(H):      # ... then one uninterrupted matmul burst
            for dt in range(DC):
                nc.tensor.matmul(
                    ps_op[dt][:], lhsT=wo[0:64, h * D + dt * 128:h * D + (dt + 1) * 128],
                    rhs=ONT[h][:], start=(h == 0), stop=(h == H - 1),
                    skip_group_check=True)
        for dt in range(DC):
            nc.vector.scalar_tensor_tensor(
                x2T[dt][:], ps_op[dt][:], bias_sb[:, 6 + dt:7 + dt], y1T[dt][:],
                op0=ALU.add, op1=ALU.add)


def _phase_ffn(nc, tc, ctx, env):
    """LN2 + interleaved FFN1(silu)/FFN2 with residual."""
    onesD32, onesrow, eps1 = env["onesD32"], env["onesrow"], env["eps1"]
    bias_sb = env["bias_sb"]
    x2T, w1, w2 = env["x2T"], env["w1"], env["w2"]

    psq2 = ctx.enter_context(tc.tile_pool(name="e_sq", bufs=3))
    ptmp2 = ctx.enter_context(tc.tile_pool(name="e_tmp", bufs=3))
    prow3 = ctx.enter_context(tc.tile_pool(name="e_rows", bufs=8))
    pn2 = ctx.enter_context(tc.tile_pool(name="e_n2", bufs=DC))
    pffs = ctx.enter_context(tc.tile_pool(name="f_ffs", bufs=2))
    poutT = ctx.enter_context(tc.tile_pool(name="f_outT", bufs=1))

    n2T = [pn2.tile([128, NQ], BF16, tag="n2", name=f"n2T{i}") for i in range(DC)]
    with ExitStack() as lctx:
        pmm2 = lctx.enter_context(tc.tile_pool(name="e_mm", bufs=3, space="PSUM"))
        pbc3 = lctx.enter_context(tc.tile_pool(name="e_bc", bufs=2, space="PSUM"))
        xs = [x2T[dc][:] for dc in range(DC)]
        r_mu2, r_S2, _ = _row_stats(nc, pmm2, prow3, psq2, onesD32, eps1,
                                    xs, NQ, F32R)
        mu2_b = _bcast(nc, pbc3, onesrow, r_mu2[:], NQ, "mu2_b")
        S2_b = _bcast(nc, pbc3, onesrow, r_S2[:], NQ, "S2_b")
        for dc in range(DC):
            tmp = ptmp2.tile([128, NQ], F32, tag="tmp2")
            nc.vector.tensor_sub(tmp[:], x2T[dc][:], mu2_b[:])
            nc.vector.tensor_mul(n2T[dc][:], tmp[:], S2_b[:])

    outT = poutT.tile([128, DC * NQ], F32, tag="outT")
    with ExitStack() as fctx:
        pmmE = fctx.enter_context(tc.tile_pool(name="f_mm", bufs=2, space="PSUM"))
        pacc = fctx.enter_context(tc.tile_pool(name="f_acc", bufs=DC, space="PSUM"))
        ps_acc = [pacc.tile([128, NQ], F32, tag="acc", name=f"ps_acc{i}")
                  for i in range(DC)]
        for ft in range(FT):
            ps1 = pmmE.tile([128, NQ], F32, tag="mm", name="ps1")
            for dc in range(DC):
                nc.tensor.matmul(
                    ps1[:], lhsT=w1[:, dc * DFF + ft * 128:dc * DFF + (ft + 1) * 128],
                    rhs=n2T[dc][:], start=(dc == 0), stop=(dc == DC - 1))
            # silu(u) = u * sigmoid(u) with u = ps1 + cb1
            sig = pffs.tile([128, NQ], BF16, tag="sig")
            nc.scalar.activation(sig[:], ps1[:], AF.Sigmoid,
                                 bias=bias_sb[:, 18 + ft:19 + ft])
            ffs = pffs.tile([128, NQ], BF16, tag="ffs")
            nc.vector.scalar_tensor_tensor(ffs[:], ps1[:], bias_sb[:, 18 + ft:19 + ft],
                                           sig[:], op0=ALU.add, op1=ALU.mult)
            for dt in range(DC):
                nc.tensor.matmul(
                    ps_acc[dt][:], lhsT=w2[:, ft * D + dt * 128:ft * D + (dt + 1) * 128],
                    rhs=ffs[:], start=(ft == 0), stop=(ft == FT - 1),
                    skip_group_check=True)
        for dt in range(DC):
            nc.vector.scalar_tensor_tensor(
                outT[:, dt * NQ:(dt + 1) * NQ], ps_acc[dt][:],
                bias_sb[:, 12 + dt:13 + dt], x2T[dt][:],
                op0=ALU.add, op1=ALU.add)
    nc.sync.dma_start(env["out"][:], outT[:])


def build_program():
    nc = bacc.Bacc("TRN2")
    env = {}
    env["xp"] = nc.declare_dram_parameter("xp", [128, NCH * DC * NQ], BF16, isOutput=False)
    env["wqkvP"] = nc.declare_dram_parameter("wqkvP", [128, 3 * 4608], BF16, isOutput=False)
    env["mskr"] = nc.declare_dram_parameter("mskr", [2, L], BF16, isOutput=False)
    env["qg"] = nc.declare_dram_parameter("qg", [2, NQ], BF16, isOutput=False)
    env["woutP"] = nc.declare_dram_parameter("woutP", [64, H * D], BF16, isOutput=False)
    env["w1P"] = nc.declare_dram_parameter("w1P", [128, DC * DFF], BF16, isOutput=False)
    env["w2P"] = nc.declare_dram_parameter("w2P", [128, FT * D], BF16, isOutput=False)
    biasP = nc.declare_dram_parameter("biasP", [128, 42], F32, isOutput=False)
    env["out"] = nc.declare_dram_parameter("out", [128, DC * NQ], F32, isOutput=True)

    with tile.TileContext(nc) as tc, ExitStack() as top:
        pc = top.enter_context(tc.tile_pool(name="const", bufs=1))
        px2 = top.enter_context(tc.tile_pool(name="x2p", bufs=DC))

        onesf = pc.tile([128, 1], F32, tag="onesf")
        nc.vector.memset(onesf[:], 1.0 / D)
        ones = pc.tile([128, 1], BF16, tag="ones")
        nc.vector.tensor_copy(ones[:], onesf[:])
        onesD32 = pc.tile([128, 1], F32R, tag="ones32")
        nc.vector.tensor_copy(onesD32[:], onesf[:])
        onesrow = pc.tile([1, 128], F32, tag="onesrow")
        nc.vector.memset(onesrow[:], 1.0)
        onesrow64 = pc.tile([1, 64], BF16, tag="onesrow64")
        nc.vector.tensor_copy(onesrow64[:], onesrow[0:1, 0:64])
        eps1 = pc.tile([1, 1], F32, tag="eps1")
        nc.vector.memset(eps1[:], EPS)
        vcolf = pc.tile([128, H], F32, tag="vcolf")
        nc.vector.memset(vcolf[:], 1.0)
        vcolb = pc.tile([128, H], BF16, tag="vcolb")
        nc.vector.tensor_copy(vcolb[:], vcolf[:])
        env["vcolb"] = vcolb
        bias_sb = pc.tile([128, 42], F32, tag="bias")
        nc.sync.dma_start(bias_sb[:], biasP[:])
        env.update(ones=ones, onesD32=onesD32, onesrow=onesrow,
                   onesrow64=onesrow64, eps1=eps1, bias_sb=bias_sb)

        env["x2T"] = [px2.tile([128, NQ], F32R, tag="x2", name=f"x2T{i}")
                      for i in range(DC)]

        with ExitStack() as mid:
            pkt = mid.enter_context(tc.tile_pool(name="ktp", bufs=H))
            pqt = mid.enter_context(tc.tile_pool(name="qtp", bufs=H))
            pva = mid.enter_context(tc.tile_pool(name="vap", bufs=NJT))
            py1 = mid.enter_context(tc.tile_pool(name="y1p", bufs=DC))
            env["KT"] = [pkt.tile([66, L], BF16, tag="kt", name=f"KT{i}")
                         for i in range(H)]
            env["QT"] = [pqt.tile([66, NQ], BF16, tag="qt", name=f"QT{i}")
                         for i in range(H)]
            env["VA"] = [pva.tile([128, H * 65], BF16, tag="va", name=f"VA{i}")
                         for i in range(NJT)]
            env["y1T"] = [py1.tile([128, NQ], BF16, tag="y1", name=f"y1T{i}")
                          for i in range(DC)]

            with ExitStack() as ctx:
                _phase_ab(nc, tc, ctx, env)

            # FFN weight pools open late: during phase AB they would starve
            # SBUF; w2's opens mid-attention. Both outlive the attention scope.
            env["pw12"] = mid.enter_context(tc.tile_pool(name="w12p", bufs=1))
            env["mid"] = mid

            with ExitStack() as ctx:
                _phase_attn(nc, tc, ctx, env)

            with ExitStack() as ctx:
                _phase_ffn(nc, tc, ctx, env)

    nc.finalize()
    return nc


_NC = None


def _get_nc():
    global _NC
    if _NC is None:
        _NC = build_program()
    return _NC


def _host_prepare(inputs):
    """Fold constants and lay out per-core input maps (pure layout work)."""
    f32 = np.float32
    x = np.asarray(inputs["x"], f32)
    memory = np.asarray(inputs["memory"], f32)
    w_qkv = np.asarray(inputs["w_qkv"], f32)
    w_out = np.asarray(inputs["w_out"], f32)
    b_out = np.asarray(inputs["b_out"], f32)
    g_att = np.asarray(inputs["ln_att_g"], f32)
    b_att = np.asarray(inputs["ln_att_b"], f32)
    g2 = np.asarray(inputs["ln2_g"], f32)
    bb2 = np.asarray(inputs["ln2_b"], f32)
    w1 = np.asarray(inputs["w1"], f32)
    b1 = np.asarray(inputs["b1"], f32)
    w2 = np.asarray(inputs["w2"], f32)
    b2v = np.asarray(inputs["b2"], f32)

    qscale = f32(64 ** -0.5)
    w_qkv_eff = w_qkv * g_att[None, :]
    w_qkv_eff[:D] *= qscale
    cb_qkv = w_qkv @ b_att
    cb_q = (cb_qkv[:D] * qscale).astype(f32)
    cb_v = cb_qkv[2 * D:].astype(f32)
    b_out_eff = (b_out + w_out @ cb_v).astype(f32)
    w1_eff = w1 * g2[None, :]
    cb1_eff = (w1 @ bb2 + b1).astype(f32)

    def cols(v):
        return np.ascontiguousarray(v.reshape(-1, 128).T)

    biasP = np.zeros((128, 42), f32)
    biasP[:, 0:6] = cols(cb_q)
    biasP[:, 6:12] = cols(b_out_eff)
    biasP[:, 12:18] = cols(b2v)
    biasP[:, 18:42] = cols(cb1_eff)

    def packP(wT, ncol):
        # [D_in, ncol] -> [128, (D_in/128)*ncol] partition-packed bf16
        return np.ascontiguousarray(
            wT.reshape(-1, 128, ncol).transpose(1, 0, 2).reshape(128, -1)
        ).astype(NPBF16)

    wq_T = np.ascontiguousarray(w_qkv_eff.T)       # [D, 3D]
    wqkvP = np.concatenate(
        [packP(np.ascontiguousarray(wq_T[:, D:2 * D]), D),      # K
         packP(np.ascontiguousarray(wq_T[:, 0:D]), D),          # Q
         packP(np.ascontiguousarray(wq_T[:, 2 * D:3 * D]), D)], # V
        axis=1)
    woutP = np.ascontiguousarray(
        w_out.T.reshape(H, 64, D).transpose(1, 0, 2).reshape(64, H * D)
    ).astype(NPBF16)

    shared = {
        "wqkvP": wqkvP,
        "woutP": woutP,
        "w1P": packP(np.ascontiguousarray(w1_eff.T), DFF),
        "w2P": packP(np.ascontiguousarray(w2.T), D),
        "biasP": biasP,
    }

    perm0 = np.concatenate([np.arange(0, T), np.arange(Q0, L), np.arange(T, Q0)])
    in_maps = []
    for c in range(NCORES):
        b, hf = divmod(c, 2)
        x_aug = np.concatenate([memory[b, :T], x[b]], axis=0)      # [L, D]
        old = perm0 if hf == 0 else np.arange(L)
        xa = x_aug[old]
        # [p, ci*6*392 + dc*392 + q] = xa[ci*392+q, dc*128+p]
        xp = np.ascontiguousarray(
            xa.T.reshape(DC, 128, NCH, NQ).transpose(1, 2, 0, 3).reshape(128, -1)
        ).astype(NPBF16)
        LcA = (5 + 2 * hf) * NPATCH
        LcB = (6 + 2 * hf) * NPATCH
        mb = np.where(old < LcB, 0.0, MASKB).astype(f32)
        ma = np.where(old < LcA, 0.0, MASKB).astype(f32)
        mskr = np.stack([mb, ma - mb]).astype(NPBF16)
        qg = np.stack([np.ones(NQ, f32),
                       (np.arange(NQ) < NPATCH).astype(f32)]).astype(NPBF16)
        in_maps.append({"xp": xp, "mskr": mskr, "qg": qg, **shared})
    return in_maps


def _assemble(results):
    out = np.zeros((B, T, D), np.float32)
    for c in range(NCORES):
        b, hf = divmod(c, 2)
        fm = results[c]["out"].reshape(128, DC, NQ).transpose(1, 0, 2).reshape(D, NQ)
        out[b, hf * NQ:(hf + 1) * NQ, :] = fm.T
    return out


def kernel(**inputs):
    nc = _get_nc()
    in_maps = _host_prepare(inputs)
    res = run_bass_kernel_spmd(nc, in_maps, list(range(NCORES)))
    return _assemble(res.results)


def _ensure_ntff_hook():
    """Provide antenv.axon_hooks (absent in this image) so trace=True can
    drive NTFF capture through libaxon_pjrt.so, mirroring trn_boot.py."""
    import contextlib
    import ctypes
    import types

    try:
        from antenv.axon_hooks import get_axon_ntff_profile_hook  # noqa: F401
        return
    except ImportError:
        pass
    import antenv

    so_path = "/opt/axon/libaxon_pjrt.so"
    lib = ctypes.CDLL(so_path)
    if not hasattr(lib, "axon_start_nrt_profile"):
        raise RuntimeError("libaxon_pjrt.so lacks NTFF profile symbols")
    lib.axon_start_nrt_profile.argtypes = [ctypes.POINTER(ctypes.c_int64),
                                           ctypes.c_size_t]
    lib.axon_start_nrt_profile.restype = ctypes.c_int64
    lib.axon_stop_nrt_profile.argtypes = [ctypes.c_char_p]
    lib.axon_stop_nrt_profile.restype = ctypes.c_int64

    @contextlib.contextmanager
    def _hook(output_dir, device_ids):
        import jax
        jax.devices()
        if device_ids:
            ids = (ctypes.c_int64 * len(device_ids))(*device_ids)
            rc = lib.axon_start_nrt_profile(ids, len(device_ids))
        else:
            rc = lib.axon_start_nrt_profile(None, 0)
        if rc != 0:
            raise RuntimeError(f"axon_start_nrt_profile rc={rc}")
        try:
            yield
        finally:
            n = lib.axon_stop_nrt_profile(str(output_dir).encode())
            print(f"ntff profile: {n} file(s) written to {output_dir}",
                  file=sys.stderr)

    box = {"h": _hook}
    mod = types.ModuleType("antenv.axon_hooks")
    mod.set_axon_ntff_profile_hook = lambda h: box.__setitem__("h", h)
    mod.get_axon_ntff_profile_hook = lambda: box["h"]
    sys.modules["antenv.axon_hooks"] = mod
    antenv.axon_hooks = mod


def kernel_traced(**inputs):
    """Like kernel() but with NTFF profiling; returns (out, exec_time_ns)."""
    import tempfile

    from concourse import bass_utils as _bu
    _ensure_ntff_hook()
    _bu.upload_artifacts = lambda tmpdir: f"local:{tmpdir}"  # no bucket creds here
    nc = _get_nc()
    in_maps = _host_prepare(inputs)
    tmpdir = tempfile.mkdtemp(prefix="ntff_")
    res = run_bass_kernel_spmd(nc, in_maps, list(range(NCORES)), trace=True,
                               tmpdir=tmpdir)
    return _assemble(res.results), res.exec_time_ns
